# revision 1
# baseline (speedup 1.0000x reference)
"""GCN classifier (GCNConv + LayerNorm + ReLU + Linear) on 8 Trainium2 NeuronCores.

Strategy (self-contained; sized for N=100000, E=1600000, 128 ch, 16 classes):
  out = LN((A @ x) @ W1 + b1).relu() @ Wfc + bfc,  A = normalized adjacency.
  - Host: add self-loops, compute per-edge norm = dinv[src]*w*dinv[dst];
    assign destination nodes to 784 tiles of 128 slots, LPT-balanced so every
    tile fits a static per-source-bank chunk profile; 98 tiles per core.
  - Device (per core): for each group of tiles, dma_gather x[src] rows (one
    call per 25000-row source bank), segment-sum each tile via one-hot
    matmuls on TensorE accumulating in PSUM, then W1 matmul, LayerNorm,
    ReLU, transpose, Wfc matmul. One bulk output store at the end.
  - Host: concatenate per-core outputs and un-permute node rows.
"""
import os
import heapq
import numpy as np

N_NODES = 100000
IN_CH = 128
HIDDEN = 128
NUM_CLASSES = 16
LN_EPS = 1e-5
N_CORES = 8
P = 128
BANK = 25000
NBANK = 4
GS = 4  # tiles per gather group

LAST_RESULTS = None
_PROGRAM_CACHE = {}


# ----------------------------------------------------------------------------
# host-side preprocessing
# ----------------------------------------------------------------------------
def _preprocess(edge_index, edge_weight):
    src = np.asarray(edge_index[0], dtype=np.int64)
    dst = np.asarray(edge_index[1], dtype=np.int64)
    w = np.asarray(edge_weight, dtype=np.float32)
    N = N_NODES
    loop = np.arange(N, dtype=np.int64)
    src = np.concatenate([src, loop])
    dst = np.concatenate([dst, loop])
    w = np.concatenate([w, np.ones(N, dtype=np.float32)])

    deg = np.bincount(dst, weights=w.astype(np.float64), minlength=N).astype(np.float32)
    dinv = np.where(deg > 0, 1.0 / np.sqrt(deg), 0.0).astype(np.float32)
    norm = (dinv[src] * w * dinv[dst]).astype(np.float32)

    # --- balanced node->tile assignment (LPT, 128-node cap per tile) ---
    cnt = np.bincount(dst, minlength=N).astype(np.int64)
    TILES = ((N + P - 1) // P + N_CORES - 1) // N_CORES * N_CORES
    while TILES * P < N:
        TILES += N_CORES
    order = np.argsort(-cnt, kind="stable")
    heap = [(0, t) for t in range(TILES)]
    heapq.heapify(heap)
    node_cnt = np.zeros(TILES, dtype=np.int64)
    edge_sum = np.zeros(TILES, dtype=np.int64)
    node_tile = np.empty(N, dtype=np.int64)
    node_slot = np.empty(N, dtype=np.int64)
    for nd in order:
        while True:
            s, t = heapq.heappop(heap)
            if node_cnt[t] < P:
                break
        node_tile[nd] = t
        node_slot[nd] = node_cnt[t]
        node_cnt[t] += 1
        edge_sum[t] += cnt[nd]
        if node_cnt[t] < P:
            heapq.heappush(heap, (edge_sum[t], t))

    TPC = TILES // N_CORES

    # --- per-(tile,bank) groups and static chunk profile ---
    et = node_tile[dst]                      # tile of each edge
    eb = src // BANK                         # source bank of each edge
    cell = np.zeros((TILES, NBANK), dtype=np.int64)
    np.add.at(cell, (et, eb), 1)
    K = (-(-cell // P)).max(axis=0)          # static chunks per bank
    Koff = np.zeros(NBANK + 1, dtype=np.int64)
    np.cumsum(K, out=Koff[1:])
    CH = int(Koff[-1])                       # chunks per tile

    # position of each edge within its (tile, bank) cell
    keys = et * NBANK + eb
    eorder = np.argsort(keys, kind="stable")
    keys_s = keys[eorder]
    gs2 = np.zeros(TILES * NBANK + 1, dtype=np.int64)
    np.cumsum(np.bincount(keys_s, minlength=TILES * NBANK), out=gs2[1:])
    pos = np.arange(len(keys_s)) - gs2[keys_s]

    src_s = src[eorder]
    dst_s = dst[eorder]
    et_s = et[eorder]
    eb_s = eb[eorder]
    norm_s = norm[eorder]

    kk = pos // P          # chunk within (tile,bank)
    lane = pos % P
    assert (kk < K[eb_s]).all(), "bank profile overflow; increase capacity"

    tl = et_s % TPC        # core-local tile
    core = et_s // TPC
    g = tl // GS
    j = tl % GS
    Sg = np.minimum(GS, TPC - g * GS)
    col_local = g * GS * CH + Sg * Koff[eb_s] + j * K[eb_s] + kk
    col = core * TPC * CH + col_local      # global metadata chunk column

    TOTC = TILES * CH
    norm_all = np.zeros((P, TOTC), dtype=np.float32)
    dstl_all = np.zeros((P, TOTC), dtype=np.float32)
    norm_all[lane, col] = norm_s
    dstl_all[lane, col] = node_slot[dst_s].astype(np.float32)

    idx16 = np.zeros((16, TOTC * 8), dtype=np.int16)
    idx16[lane % 16, col * 8 + lane // 16] = (src_s % BANK).astype(np.int16)
    idx_all = np.tile(idx16, (8, 1))

    return dict(
        idx_all=idx_all, norm_all=norm_all, dstl_all=dstl_all,
        node_tile=node_tile, node_slot=node_slot,
        TILES=TILES, CHUNKS=CH, TPC=TPC,
        K=tuple(int(k) for k in K),
    )


def _groups(TPC):
    out = []
    base = 0
    t = 0
    while t < TPC:
        s = min(GS, TPC - t)
        out.append((s, base))
        base += s * 0 + s  # tiles consumed
        t += s
    return [(s, i * GS) for i, (s, _) in enumerate(out)]


# ----------------------------------------------------------------------------
# device program
# ----------------------------------------------------------------------------
def _build_program(TPC, CH, K):
    from contextlib import ExitStack
    import concourse.bass as bass
    import concourse.tile as tile
    from concourse import bacc, mybir

    f32 = mybir.dt.float32
    i16 = mybir.dt.int16
    NCOLS = TPC * CH
    Koff = [0]
    for k in K:
        Koff.append(Koff[-1] + k)

    nc = bacc.Bacc("TRN2", target_bir_lowering=False, debug=False,
                   num_devices=N_CORES)
    xb = [nc.dram_tensor(f"xb{b}", [BANK, IN_CH], f32, kind="ExternalInput").ap()
          for b in range(NBANK)]
    idx_d = nc.dram_tensor("idx", [P, NCOLS * 8], i16, kind="ExternalInput").ap()
    dstl_d = nc.dram_tensor("dstl", [P, NCOLS], f32, kind="ExternalInput").ap()
    norm_d = nc.dram_tensor("normv", [P, NCOLS], f32, kind="ExternalInput").ap()
    w1_d = nc.dram_tensor("W1", [IN_CH, HIDDEN], f32, kind="ExternalInput").ap()
    wfc_d = nc.dram_tensor("Wfc", [HIDDEN, NUM_CLASSES], f32, kind="ExternalInput").ap()
    b1_d = nc.dram_tensor("b1", [1, HIDDEN], f32, kind="ExternalInput").ap()
    lng_d = nc.dram_tensor("ln_g", [1, HIDDEN], f32, kind="ExternalInput").ap()
    lnb_d = nc.dram_tensor("ln_b", [1, HIDDEN], f32, kind="ExternalInput").ap()
    bfc_d = nc.dram_tensor("bfc", [1, NUM_CLASSES], f32, kind="ExternalInput").ap()
    iota_d = nc.dram_tensor("iota", [1, P], f32, kind="ExternalInput").ap()
    idm_d = nc.dram_tensor("idm", [P, P], f32, kind="ExternalInput").ap()
    out_d = nc.dram_tensor("out", [TPC * P, NUM_CLASSES], f32,
                           kind="ExternalOutput").ap()

    def bcast(src_ap, parts=P):
        return bass.AP(tensor=src_ap.tensor, offset=src_ap.offset,
                       ap=[[0, parts]] + list(src_ap.ap[1:]))

    AL = mybir.AluOpType
    AF = mybir.ActivationFunctionType

    with tile.TileContext(nc) as tc, ExitStack() as ctx:
        consts = ctx.enter_context(tc.tile_pool(name="consts", bufs=1))
        gpool = ctx.enter_context(tc.tile_pool(name="gather", bufs=2))
        ohpool = ctx.enter_context(tc.tile_pool(name="onehot", bufs=4))
        sp = ctx.enter_context(tc.tile_pool(name="work", bufs=4))
        statp = ctx.enter_context(tc.tile_pool(name="stats", bufs=8))
        pp_ps = ctx.enter_context(tc.tile_pool(name="pp_ps", bufs=2, space="PSUM"))
        agg_ps = ctx.enter_context(tc.tile_pool(name="agg_ps", bufs=2, space="PSUM"))
        tr_ps = ctx.enter_context(tc.tile_pool(name="tr_ps", bufs=2, space="PSUM"))
        fc_ps = ctx.enter_context(tc.tile_pool(name="fc_ps", bufs=2, space="PSUM"))

        W1_s = consts.tile([IN_CH, HIDDEN], f32)
        nc.sync.dma_start(W1_s[:], w1_d[:])
        Wfc_s = consts.tile([HIDDEN, NUM_CLASSES], f32)
        nc.sync.dma_start(Wfc_s[:], wfc_d[:])
        B1 = consts.tile([P, HIDDEN], f32)
        nc.sync.dma_start(B1[:], bcast(b1_d))
        LNG = consts.tile([P, HIDDEN], f32)
        nc.sync.dma_start(LNG[:], bcast(lng_d))
        LNB = consts.tile([P, HIDDEN], f32)
        nc.sync.dma_start(LNB[:], bcast(lnb_d))
        BFC = consts.tile([P, NUM_CLASSES], f32)
        nc.sync.dma_start(BFC[:], bcast(bfc_d))
        IOTA = consts.tile([P, P], f32)
        nc.sync.dma_start(IOTA[:], bcast(iota_d))
        ident = consts.tile([P, P], f32)
        nc.sync.dma_start(ident[:], idm_d[:])
        eps_t = consts.tile([P, 1], f32)
        nc.vector.memset(eps_t[:], LN_EPS)

        idx_s = consts.tile([P, NCOLS * 8], i16)
        nc.sync.dma_start(idx_s[:], idx_d[:])
        dstl_s = consts.tile([P, NCOLS], f32)
        nc.sync.dma_start(dstl_s[:], dstl_d[:])
        norm_s = consts.tile([P, NCOLS], f32)
        nc.sync.dma_start(norm_s[:], norm_d[:])

        out_acc = consts.tile([P, TPC * NUM_CLASSES], f32)

        t_global = 0
        for s, gbase_tile in _groups(TPC):
            gbase = gbase_tile * CH       # chunk-column base of this group
            Gg = gpool.tile([P, GS * CH, IN_CH], f32, tag="Gg")
            for b in range(NBANK):
                n = s * K[b] * P
                ccol = gbase + s * Koff[b]
                nc.gpsimd.dma_gather(
                    out_ap=Gg[:, s * Koff[b]:s * Koff[b] + s * K[b], :],
                    in_ap=xb[b][:],
                    idxs_ap=idx_s[:, ccol * 8:ccol * 8 + n // 16],
                    num_idxs=n, num_idxs_reg=n, elem_size=IN_CH,
                    single_packet=False,
                )
            for j in range(s):
                t = t_global
                t_global += 1
                Pp = pp_ps.tile([IN_CH, P], f32, space="PSUM")
                mm = 0
                for b in range(NBANK):
                    for kk in range(K[b]):
                        cig = s * Koff[b] + j * K[b] + kk
                        col = gbase + cig
                        oh = ohpool.tile([P, P], f32, tag="oh")
                        nc.vector.tensor_scalar(
                            out=oh[:], in0=IOTA[:],
                            scalar1=dstl_s[:, col:col + 1],
                            scalar2=norm_s[:, col:col + 1],
                            op0=AL.is_equal, op1=AL.mult)
                        nc.tensor.matmul(Pp[:], lhsT=Gg[:, cig, :], rhs=oh[:],
                                         start=(mm == 0), stop=(mm == CH - 1))
                        mm += 1
                Ps = sp.tile([IN_CH, P], f32, tag="Ps")
                nc.vector.tensor_copy(Ps[:], Pp[:])
                agg = agg_ps.tile([P, HIDDEN], f32, space="PSUM")
                nc.tensor.matmul(agg[:], lhsT=Ps[:], rhs=W1_s[:],
                                 start=True, stop=True)
                # LayerNorm over free dim
                t1 = sp.tile([P, HIDDEN], f32, tag="t1")
                musum = statp.tile([P, 1], f32, tag="musum")
                nc.vector.scalar_tensor_tensor(
                    out=t1[:], in0=agg[:], scalar=1.0, in1=B1[:],
                    op0=AL.mult, op1=AL.add, accum_out=musum[:])
                nc.vector.tensor_scalar_mul(musum[:], musum[:], 1.0 / HIDDEN)
                t1c = sp.tile([P, HIDDEN], f32, tag="t1c")
                nc.vector.tensor_scalar(out=t1c[:], in0=t1[:], scalar1=musum[:],
                                        scalar2=None, op0=AL.subtract)
                sq = sp.tile([P, HIDDEN], f32, tag="sq")
                varsum = statp.tile([P, 1], f32, tag="varsum")
                nc.scalar.activation(out=sq[:], in_=t1c[:], func=AF.Square,
                                     accum_out=varsum[:])
                rstd = statp.tile([P, 1], f32, tag="rstd")
                nc.scalar.activation(out=rstd[:], in_=varsum[:], func=AF.Sqrt,
                                     bias=eps_t[:], scale=1.0 / HIDDEN)
                nc.vector.reciprocal(out=rstd[:], in_=rstd[:])
                y0 = sp.tile([P, HIDDEN], f32, tag="y0")
                nc.vector.scalar_tensor_tensor(
                    out=y0[:], in0=t1c[:], scalar=rstd[:], in1=LNG[:],
                    op0=AL.mult, op1=AL.mult)
                y1 = sp.tile([P, HIDDEN], f32, tag="y1")
                nc.vector.tensor_tensor(out=y1[:], in0=y0[:], in1=LNB[:], op=AL.add)
                hr = sp.tile([P, HIDDEN], f32, tag="hr")
                nc.scalar.activation(out=hr[:], in_=y1[:], func=AF.Relu)
                hrT_ps = tr_ps.tile([HIDDEN, P], f32, space="PSUM")
                nc.tensor.transpose(out=hrT_ps[:], in_=hr[:], identity=ident[:])
                hrT = sp.tile([HIDDEN, P], f32, tag="hrT")
                nc.vector.tensor_copy(hrT[:], hrT_ps[:])
                o_ps = fc_ps.tile([P, NUM_CLASSES], f32, space="PSUM")
                nc.tensor.matmul(o_ps[:], lhsT=hrT[:], rhs=Wfc_s[:],
                                 start=True, stop=True)
                nc.vector.tensor_tensor(
                    out=out_acc[:, t * NUM_CLASSES:(t + 1) * NUM_CLASSES],
                    in0=o_ps[:], in1=BFC[:], op=AL.add)

        out_view = out_d.rearrange("(t p) c -> p t c", p=P)
        acc_view = out_acc[:].rearrange("p (t c) -> p t c", c=NUM_CLASSES)
        nc.sync.dma_start(out_view, acc_view)

    nc.compile()
    return nc


def _ensure_ntff_hook():
    import sys, types
    try:
        from antenv.axon_hooks import get_axon_ntff_profile_hook  # noqa: F401
        return
    except ImportError:
        pass
    mod = types.ModuleType("antenv.axon_hooks")
    _hook = [None]
    mod.set_axon_ntff_profile_hook = lambda h: _hook.__setitem__(0, h)
    mod.get_axon_ntff_profile_hook = lambda: _hook[0]
    sys.modules["antenv.axon_hooks"] = mod
    try:
        import antenv
        antenv.axon_hooks = mod
    except ImportError:
        pass
    try:
        from trn_agent_boot.trn_boot import _ntff_profile_via_ctypes
        mod.set_axon_ntff_profile_hook(
            _ntff_profile_via_ctypes("/opt/axon/libaxon_pjrt.so"))
    except Exception:
        pass


# ----------------------------------------------------------------------------
# entry point
# ----------------------------------------------------------------------------
def kernel(x, edge_index, edge_weight, W1, b1, ln_g, ln_b, Wfc, bfc):
    global LAST_RESULTS
    from concourse.bass_utils import run_bass_kernel_spmd

    x = np.ascontiguousarray(np.asarray(x, dtype=np.float32))
    meta = _preprocess(edge_index, edge_weight)
    TPC, CH, K = meta["TPC"], meta["CHUNKS"], meta["K"]

    key = (TPC, CH, K)
    if key not in _PROGRAM_CACHE:
        _PROGRAM_CACHE[key] = _build_program(TPC, CH, K)
    nc = _PROGRAM_CACHE[key]

    NCOLS = TPC * CH
    banks = {}
    for b in range(NBANK):
        blk = np.zeros((BANK, IN_CH), dtype=np.float32)
        seg = x[b * BANK:(b + 1) * BANK]
        blk[:len(seg)] = seg
        banks[f"xb{b}"] = blk
    common = dict(
        banks,
        W1=np.ascontiguousarray(np.asarray(W1, np.float32)),
        Wfc=np.ascontiguousarray(np.asarray(Wfc, np.float32)),
        b1=np.asarray(b1, np.float32).reshape(1, HIDDEN),
        ln_g=np.asarray(ln_g, np.float32).reshape(1, HIDDEN),
        ln_b=np.asarray(ln_b, np.float32).reshape(1, HIDDEN),
        bfc=np.asarray(bfc, np.float32).reshape(1, NUM_CLASSES),
        iota=np.arange(P, dtype=np.float32).reshape(1, P),
        idm=np.eye(P, dtype=np.float32),
    )
    in_maps = []
    for core in range(N_CORES):
        sl = slice(core * NCOLS, (core + 1) * NCOLS)
        sl8 = slice(core * NCOLS * 8, (core + 1) * NCOLS * 8)
        in_maps.append(dict(
            common,
            idx=np.ascontiguousarray(meta["idx_all"][:, sl8]),
            dstl=np.ascontiguousarray(meta["dstl_all"][:, sl]),
            normv=np.ascontiguousarray(meta["norm_all"][:, sl]),
        ))

    trace = bool(os.environ.get("KERNEL_TRACE"))
    if trace:
        _ensure_ntff_hook()
    res = run_bass_kernel_spmd(nc, in_maps, list(range(N_CORES)), trace=trace)
    LAST_RESULTS = res

    all_rows = np.concatenate([res.results[c]["out"] for c in range(N_CORES)],
                              axis=0)
    rows = meta["node_tile"] * P + meta["node_slot"]
    return np.ascontiguousarray(all_rows[rows])



# revision 6
# speedup vs baseline: 3.2279x; 3.2279x over previous
"""GCN classifier (GCNConv + LayerNorm + ReLU + Linear) on 8 Trainium2 NeuronCores.

v2 strategy (self-contained; sized for N=100000, E=1600000, 128 ch, 16 classes):
  out = LN((A @ x) @ W1 + b1).relu() @ Wfc + bfc,  A = normalized adjacency.

  Profiling insights driving this design (vs v1 baseline @ 2.48ms):
  - SWDGE descriptor generation on the Q7 cores is ~8ns/descriptor and was
    2.0ms serial on one core pair. Fix: num_swdge_queues=4, one gather call
    per source bank on its own queue_num -> 4 Q7 pairs generate in parallel.
  - DVE tensor_scalar/copy can enter 2-port perf mode which takes an
    exclusive lock on the SBUF port shared with GPSIMD -> one-hot builds
    were blocking descriptor generation (and vice versa), 4.2ms of DVE time.
    Fix: build one-hot slabs with tensor_tensor (never contends) using
    stride-0 repeat APs; PSUM evacuations / scaling moved to the ACT engine
    (own SBUF port).
  - fp32 matmuls are 4 cycles/row on the PE; fp16 is 1. Everything on the
    matmul path is fp16 (tolerance is 2e-2; fp16 keeps us ~1e-3).
  - Padding trimmed: nodes are packed into 888 tiles so every (tile, bank)
    cell fits exactly K=4 chunks of 128 edges (~7% pad vs ~30%); self-loops
    are not gathered at all - they stream as dense 128-row blocks from a
    tile-permuted fp16 copy of x via HWDGE (free of Q7 descriptor cost).
  - LayerNorm mean comes free as an extra (negated row-mean) column of the
    W1 matmul; LN affine + ReLU fold into one ACT op in transposed layout.
"""
import heapq
import os

import numpy as np

N_NODES = 100000
IN_CH = 128
HIDDEN = 128
NUM_CLASSES = 16
LN_EPS = 1e-5
N_CORES = 8
P = 128
BANK = 25000
NBANK = 4
K = 4                 # chunks per (tile, bank)
CPT = NBANK * K + 1   # chunks per tile (16 gather + 1 dense self block)
CELLCAP = K * P       # max edges per (tile, bank)
TILES = 888
TPC = TILES // N_CORES
GS = 8                # tiles per gather group

LAST_RESULTS = None
_PROGRAM_CACHE = {}


def _groups():
    out = []
    t = 0
    while t < TPC:
        s = min(GS, TPC - t)
        out.append((s, t))
        t += s
    return out


def _call_col_bases():
    """Column base (in 16-wide int16 idx columns) of each (group, bank) gather."""
    bases = []
    run = 0
    for s, _ in _groups():
        row = []
        for _b in range(NBANK):
            row.append(run)
            run += s * K * P // 16
        bases.append(row)
    return bases, run


# ----------------------------------------------------------------------------
# host-side preprocessing
# ----------------------------------------------------------------------------
def _assign_tiles(dst, eb, cnt_nb):
    """LPT-pack nodes into TILES tiles (<=128 nodes each), then repair so every
    (tile, bank) cell holds <= CELLCAP edges."""
    N = N_NODES
    cnt = cnt_nb.sum(axis=1)
    order = np.argsort(-cnt, kind="stable")
    heap = [(0, t) for t in range(TILES)]
    heapq.heapify(heap)
    node_cnt = np.zeros(TILES, dtype=np.int64)
    edge_sum = np.zeros(TILES, dtype=np.int64)
    node_tile = np.empty(N, dtype=np.int64)
    for nd in order:
        while True:
            s, t = heapq.heappop(heap)
            if node_cnt[t] < P:
                break
        node_tile[nd] = t
        node_cnt[t] += 1
        edge_sum[t] += cnt[nd]
        if node_cnt[t] < P:
            heapq.heappush(heap, (edge_sum[t], t))

    # repair per-bank overflows
    for _ in range(64):
        cell = np.zeros((TILES, NBANK), dtype=np.int64)
        np.add.at(cell, (node_tile[dst], eb), 1)
        over = np.argwhere(cell > CELLCAP)
        if len(over) == 0:
            break
        node_cnt = np.bincount(node_tile, minlength=TILES)
        for t, b in over:
            excess = cell[t, b] - CELLCAP
            if excess <= 0:
                continue
            nodes_t = np.where(node_tile == t)[0]
            cand = nodes_t[np.argsort(-cnt_nb[nodes_t, b], kind="stable")]
            for nd in cand:
                if excess <= 0:
                    break
                c_nd = cnt_nb[nd]
                if c_nd[b] == 0:
                    break
                ok = (node_cnt < P) & ((cell + c_nd[None, :]) <= CELLCAP).all(axis=1)
                ok[t] = False
                if not ok.any():
                    continue
                cand_t2 = np.where(ok)[0]
                t2 = cand_t2[np.argmin(cell[cand_t2].sum(axis=1))]
                node_tile[nd] = t2
                cell[t] -= c_nd
                cell[t2] += c_nd
                node_cnt[t] -= 1
                node_cnt[t2] += 1
                excess = cell[t, b] - CELLCAP
    else:
        raise RuntimeError("tile repair did not converge")

    # compact slots within each tile
    order2 = np.argsort(node_tile, kind="stable")
    tile_sorted = node_tile[order2]
    starts = np.zeros(TILES + 1, dtype=np.int64)
    np.cumsum(np.bincount(tile_sorted, minlength=TILES), out=starts[1:])
    node_slot = np.empty(N, dtype=np.int64)
    node_slot[order2] = np.arange(N) - starts[tile_sorted]
    assert (node_slot < P).all()
    return node_tile, node_slot


def _preprocess(edge_index, edge_weight):
    src = np.asarray(edge_index[0], dtype=np.int64)
    dst = np.asarray(edge_index[1], dtype=np.int64)
    w = np.asarray(edge_weight, dtype=np.float32)
    N = N_NODES

    deg = np.bincount(dst, weights=w.astype(np.float64), minlength=N) + 1.0
    dinv = (1.0 / np.sqrt(deg)).astype(np.float32)
    norm = (dinv[src] * w * dinv[dst]).astype(np.float32)
    selfnorm = (dinv.astype(np.float64) ** 2).astype(np.float32)  # 1/deg

    eb = src // BANK
    cnt_nb = np.zeros((N, NBANK), dtype=np.int64)
    np.add.at(cnt_nb, (dst, eb), 1)
    node_tile, node_slot = _assign_tiles(dst, eb, cnt_nb)

    # per-edge (tile, bank) cell position
    et = node_tile[dst]
    keys = et * NBANK + eb
    eorder = np.argsort(keys, kind="stable")
    keys_s = keys[eorder]
    cum = np.zeros(TILES * NBANK + 1, dtype=np.int64)
    np.cumsum(np.bincount(keys_s, minlength=TILES * NBANK), out=cum[1:])
    pos = np.arange(len(keys_s)) - cum[keys_s]
    kk = pos // P
    lane = pos % P
    assert (kk < K).all(), "cell overflow after repair"

    src_s = src[eorder]
    dst_s = dst[eorder]
    et_s = et[eorder]
    eb_s = eb[eorder]
    norm_s = norm[eorder]

    # tile-major metadata: column = tile*CPT + (bank*K + kk); self chunk at +16
    MCOLS = TILES * CPT
    mcol = et_s * CPT + eb_s * K + kk
    dstl_all = np.zeros((P, MCOLS), dtype=np.float16)
    norm_all = np.zeros((P, MCOLS), dtype=np.float16)
    dstl_all[lane, mcol] = node_slot[dst_s].astype(np.float16)
    norm_all[lane, mcol] = norm_s.astype(np.float16)
    scol = node_tile * CPT + NBANK * K
    dstl_all[node_slot, scol] = node_slot.astype(np.float16)
    norm_all[node_slot, scol] = selfnorm.astype(np.float16)

    # gather indices, call-major: per core, per (group, bank) call,
    # within call linear i = (j*K + kk)*128 + lane
    bases, IDXC = _call_col_bases()
    core = et_s // TPC
    tl = et_s % TPC
    g = tl // GS
    j = tl % GS
    cb = np.asarray([[bases[gi][bi] for bi in range(NBANK)]
                     for gi in range(len(bases))], dtype=np.int64)
    i_lin = (j * K + kk) * P + lane
    col16 = core * IDXC + cb[g, eb_s] + i_lin // 16
    row16 = i_lin % 16
    idx16 = np.zeros((16, N_CORES * IDXC), dtype=np.int16)
    idx16[row16, col16] = (src_s % BANK).astype(np.int16)
    idx_all = np.tile(idx16, (8, 1))

    return dict(
        idx_all=idx_all, norm_all=norm_all, dstl_all=dstl_all,
        node_tile=node_tile, node_slot=node_slot, IDXC=IDXC,
    )


# ----------------------------------------------------------------------------
# device program
# ----------------------------------------------------------------------------
def _build_program():
    from contextlib import ExitStack
    import concourse.bass as bass
    import concourse.tile as tile
    from concourse import bacc, mybir

    f32 = mybir.dt.float32
    f16 = mybir.dt.float16
    i16 = mybir.dt.int16
    H = HIDDEN
    MC = TPC * CPT
    bases, IDXC = _call_col_bases()

    nc = bacc.Bacc("TRN2", target_bir_lowering=False, debug=False,
                   num_devices=N_CORES, num_swdge_queues=4)
    xb = [nc.dram_tensor(f"xb{b}", [BANK, IN_CH], f16, kind="ExternalInput").ap()
          for b in range(NBANK)]
    xperm_d = nc.dram_tensor("xperm", [TPC * P, IN_CH], f16, kind="ExternalInput").ap()
    idx_d = nc.dram_tensor("idx", [P, IDXC], i16, kind="ExternalInput").ap()
    dstl_d = nc.dram_tensor("dstl", [P, MC], f16, kind="ExternalInput").ap()
    norm_d = nc.dram_tensor("normv", [P, MC], f16, kind="ExternalInput").ap()
    w1_d = nc.dram_tensor("W1aug", [IN_CH, H + 1], f16, kind="ExternalInput").ap()
    wfc_d = nc.dram_tensor("Wfc", [H, NUM_CLASSES], f16, kind="ExternalInput").ap()
    c1_d = nc.dram_tensor("c1", [1, H], f32, kind="ExternalInput").ap()
    lng_d = nc.dram_tensor("ln_g", [H, 1], f32, kind="ExternalInput").ap()
    lnb_d = nc.dram_tensor("ln_b", [H, 1], f32, kind="ExternalInput").ap()
    bfc_d = nc.dram_tensor("bfc", [1, NUM_CLASSES], f32, kind="ExternalInput").ap()
    iota_d = nc.dram_tensor("iota", [1, P], f16, kind="ExternalInput").ap()
    idm_d = nc.dram_tensor("idm", [P, P], f16, kind="ExternalInput").ap()
    out_d = nc.dram_tensor("out", [TPC * P, NUM_CLASSES], f32,
                           kind="ExternalOutput").ap()

    def bcast(src_ap, parts=P):
        return bass.AP(tensor=src_ap.tensor, offset=src_ap.offset,
                       ap=[[0, parts]] + list(src_ap.ap[1:]))

    def rep_inner(ap2d, n):
        """[p, c] -> [p, c, n] with the last dim broadcast (stride 0)."""
        return bass.AP(tensor=ap2d.tensor, offset=ap2d.offset,
                       ap=list(ap2d.ap) + [[0, n]])

    def rep_mid(ap2d, n):
        """[p, q] -> [p, n, q] with the middle dim broadcast (stride 0)."""
        a = list(ap2d.ap)
        return bass.AP(tensor=ap2d.tensor, offset=ap2d.offset,
                       ap=[a[0], [0, n], a[1]])

    AL = mybir.AluOpType
    AF = mybir.ActivationFunctionType

    with tile.TileContext(nc) as tc, ExitStack() as ctx:
        consts = ctx.enter_context(tc.tile_pool(name="consts", bufs=1))
        gpool = ctx.enter_context(tc.tile_pool(name="gather", bufs=2))
        ohp = ctx.enter_context(tc.tile_pool(name="onehot", bufs=4))
        sp = ctx.enter_context(tc.tile_pool(name="work", bufs=4))
        t1p = ctx.enter_context(tc.tile_pool(name="t1c", bufs=2 * GS))
        statp = ctx.enter_context(tc.tile_pool(name="stats", bufs=4 * GS))
        pp_ps = ctx.enter_context(tc.tile_pool(name="pp_ps", bufs=2, space="PSUM"))
        agg_ps = ctx.enter_context(tc.tile_pool(name="agg_ps", bufs=2, space="PSUM"))
        tr_ps = ctx.enter_context(tc.tile_pool(name="tr_ps", bufs=2, space="PSUM"))
        fc_ps = ctx.enter_context(tc.tile_pool(name="fc_ps", bufs=2, space="PSUM"))

        W1s = consts.tile([IN_CH, H + 1], f16)
        nc.sync.dma_start(W1s[:], w1_d[:])
        Wfcs = consts.tile([H, NUM_CLASSES], f16)
        nc.sync.dma_start(Wfcs[:], wfc_d[:])
        C1s = consts.tile([P, H], f32)
        nc.sync.dma_start(C1s[:], bcast(c1_d))
        Gcol = consts.tile([H, 1], f32)
        nc.sync.dma_start(Gcol[:], lng_d[:])
        Bcol = consts.tile([H, 1], f32)
        nc.sync.dma_start(Bcol[:], lnb_d[:])
        BFCs = consts.tile([P, NUM_CLASSES], f32)
        nc.sync.dma_start(BFCs[:], bcast(bfc_d))
        IOTA = consts.tile([P, P], f16)
        nc.sync.dma_start(IOTA[:], bcast(iota_d))
        idents = consts.tile([P, P], f16)
        nc.sync.dma_start(idents[:], idm_d[:])
        eps_t = consts.tile([P, 1], f32)
        nc.vector.memset(eps_t[:], LN_EPS)

        idx_s = consts.tile([P, IDXC], i16)
        nc.sync.dma_start(idx_s[:], idx_d[:])
        dstl_s = consts.tile([P, MC], f16)
        nc.sync.dma_start(dstl_s[:], dstl_d[:])
        norm_s = consts.tile([P, MC], f16)
        nc.sync.dma_start(norm_s[:], norm_d[:])

        out_acc = consts.tile([P, TPC * NUM_CLASSES], f32)

        for gi, (s, tb) in enumerate(_groups()):
            Gg = gpool.tile([P, s * CPT, IN_CH], f16, tag="Gg")
            for b in range(NBANK):
                n = s * K * P
                cbase = bases[gi][b]
                nc.gpsimd.dma_gather(
                    out_ap=Gg[:, b * s * K:(b + 1) * s * K, :],
                    in_ap=xb[b][:],
                    idxs_ap=idx_s[:, cbase:cbase + n // 16],
                    num_idxs=n, num_idxs_reg=n, elem_size=IN_CH,
                    single_packet=False, queue_num=b,
                )
            # dense self blocks via HWDGE
            xp_in = bass.AP(tensor=xperm_d.tensor,
                            offset=xperm_d.offset + tb * P * IN_CH,
                            ap=[[IN_CH, P], [P * IN_CH, s], [1, IN_CH]])
            nc.sync.dma_start(Gg[:, NBANK * K * s:NBANK * K * s + s, :], xp_in)

            vars_g = statp.tile([P, s], f32, tag="vars")
            t1cs = []
            for j in range(s):
                t = tb + j
                mc0 = t * CPT
                dsl = dstl_s[:, mc0:mc0 + CPT]
                nsl = norm_s[:, mc0:mc0 + CPT]
                tmp = ohp.tile([P, CPT * P], f16, tag="tmp")
                tmp3 = tmp[:].rearrange("p (c q) -> p c q", q=P)
                nc.vector.tensor_tensor(out=tmp3, in0=rep_inner(dsl, P),
                                        in1=rep_mid(IOTA[:], CPT), op=AL.is_equal)
                ohs = ohp.tile([P, CPT * P], f16, tag="ohs")
                ohs3 = ohs[:].rearrange("p (c q) -> p c q", q=P)
                nc.vector.tensor_tensor(out=ohs3, in0=tmp3,
                                        in1=rep_inner(nsl, P), op=AL.mult)

                Pp = pp_ps.tile([IN_CH, P], f32, space="PSUM")
                for c in range(CPT):
                    if c < NBANK * K:
                        gcol = (c // K) * s * K + j * K + (c % K)
                    else:
                        gcol = NBANK * K * s + j
                    nc.tensor.matmul(Pp[:], lhsT=Gg[:, gcol, :],
                                     rhs=ohs[:, c * P:(c + 1) * P],
                                     start=(c == 0), stop=(c == CPT - 1))
                Ps = sp.tile([IN_CH, P], f16, tag="Ps")
                nc.scalar.activation(out=Ps[:], in_=Pp[:], func=AF.Copy)
                agg = agg_ps.tile([P, H + 1], f32, space="PSUM")
                nc.tensor.matmul(agg[:], lhsT=Ps[:], rhs=W1s[:],
                                 start=True, stop=True)
                mu = statp.tile([P, 1], f32, tag="mu")
                nc.scalar.activation(out=mu[:], in_=agg[:, H:H + 1], func=AF.Copy)
                t1c = t1p.tile([P, H], f32, tag="t1c")
                nc.vector.scalar_tensor_tensor(
                    out=t1c[:], in0=agg[:, 0:H], scalar=mu[:], in1=C1s[:],
                    op0=AL.add, op1=AL.add)
                sq = sp.tile([P, H], f32, tag="sq")
                nc.scalar.activation(out=sq[:], in_=t1c[:], func=AF.Square,
                                     accum_out=vars_g[:, j:j + 1])
                t1cs.append(t1c)

            stdg = statp.tile([P, s], f32, tag="std")
            nc.scalar.activation(out=stdg[:], in_=vars_g[:], func=AF.Sqrt,
                                 bias=eps_t[:], scale=1.0 / H)
            rstd = statp.tile([P, s], f32, tag="rstd")
            nc.vector.reciprocal(out=rstd[:], in_=stdg[:])

            fc = fc_ps.tile([P, s * NUM_CLASSES], f32, space="PSUM")
            for j in range(s):
                t1n = sp.tile([P, H], f16, tag="t1n")
                nc.scalar.activation(out=t1n[:], in_=t1cs[j][:], func=AF.Copy,
                                     scale=rstd[:, j:j + 1])
                yT = tr_ps.tile([H, P], f16, space="PSUM")
                nc.tensor.transpose(out=yT[:], in_=t1n[:], identity=idents[:])
                hrT = sp.tile([H, P], f16, tag="hrT")
                nc.scalar.activation(out=hrT[:], in_=yT[:], func=AF.Relu,
                                     scale=Gcol[:], bias=Bcol[:])
                nc.tensor.matmul(fc[:, j * NUM_CLASSES:(j + 1) * NUM_CLASSES],
                                 lhsT=hrT[:], rhs=Wfcs[:], start=True, stop=True)

            oslice = out_acc[:, tb * NUM_CLASSES:(tb + s) * NUM_CLASSES]
            o3 = oslice.rearrange("p (t c) -> p t c", c=NUM_CLASSES)
            f3 = fc[:].rearrange("p (t c) -> p t c", c=NUM_CLASSES)
            nc.vector.tensor_tensor(out=o3, in0=f3,
                                    in1=rep_mid(BFCs[:], s), op=AL.add)

        out_view = out_d.rearrange("(t p) c -> p t c", p=P)
        acc_view = out_acc[:].rearrange("p (t c) -> p t c", c=NUM_CLASSES)
        nc.sync.dma_start(out_view, acc_view)

    nc.compile()
    return nc


def _ensure_ntff_hook():
    import sys, types
    try:
        from antenv.axon_hooks import get_axon_ntff_profile_hook  # noqa: F401
        return
    except ImportError:
        pass
    mod = types.ModuleType("antenv.axon_hooks")
    _hook = [None]
    mod.set_axon_ntff_profile_hook = lambda h: _hook.__setitem__(0, h)
    mod.get_axon_ntff_profile_hook = lambda: _hook[0]
    sys.modules["antenv.axon_hooks"] = mod
    try:
        import antenv
        antenv.axon_hooks = mod
    except ImportError:
        pass
    try:
        from trn_agent_boot.trn_boot import _ntff_profile_via_ctypes
        mod.set_axon_ntff_profile_hook(
            _ntff_profile_via_ctypes("/opt/axon/libaxon_pjrt.so"))
    except Exception:
        pass


# ----------------------------------------------------------------------------
# entry point
# ----------------------------------------------------------------------------
def kernel(x, edge_index, edge_weight, W1, b1, ln_g, ln_b, Wfc, bfc):
    global LAST_RESULTS
    from concourse.bass_utils import run_bass_kernel_spmd

    x16 = np.asarray(x, dtype=np.float32).astype(np.float16)
    meta = _preprocess(edge_index, edge_weight)
    IDXC = meta["IDXC"]

    if "prog" not in _PROGRAM_CACHE:
        _PROGRAM_CACHE["prog"] = _build_program()
    nc = _PROGRAM_CACHE["prog"]

    W1f = np.asarray(W1, np.float32)
    W1aug = np.zeros((IN_CH, HIDDEN + 1), dtype=np.float16)
    W1aug[:, :HIDDEN] = W1f.astype(np.float16)
    W1aug[:, HIDDEN] = (-W1f.mean(axis=1)).astype(np.float16)
    b1f = np.asarray(b1, np.float32).reshape(-1)
    c1 = (b1f - b1f.mean()).reshape(1, HIDDEN).astype(np.float32)

    xperm = np.zeros((TILES * P, IN_CH), dtype=np.float16)
    rows = meta["node_tile"] * P + meta["node_slot"]
    xperm[rows] = x16

    banks = {}
    for b in range(NBANK):
        blk = np.zeros((BANK, IN_CH), dtype=np.float16)
        seg = x16[b * BANK:(b + 1) * BANK]
        blk[:len(seg)] = seg
        banks[f"xb{b}"] = blk

    common = dict(
        banks,
        W1aug=W1aug,
        Wfc=np.asarray(Wfc, np.float32).astype(np.float16),
        c1=c1,
        ln_g=np.asarray(ln_g, np.float32).reshape(HIDDEN, 1),
        ln_b=np.asarray(ln_b, np.float32).reshape(HIDDEN, 1),
        bfc=np.asarray(bfc, np.float32).reshape(1, NUM_CLASSES),
        iota=np.arange(P, dtype=np.float16).reshape(1, P),
        idm=np.eye(P, dtype=np.float16),
    )
    MC = TPC * CPT
    in_maps = []
    for core in range(N_CORES):
        msl = slice(core * MC, (core + 1) * MC)
        in_maps.append(dict(
            common,
            idx=np.ascontiguousarray(meta["idx_all"][:, core * IDXC:(core + 1) * IDXC]),
            dstl=np.ascontiguousarray(meta["dstl_all"][:, msl]),
            normv=np.ascontiguousarray(meta["norm_all"][:, msl]),
            xperm=np.ascontiguousarray(xperm[core * TPC * P:(core + 1) * TPC * P]),
        ))

    trace = bool(os.environ.get("KERNEL_TRACE"))
    if trace:
        _ensure_ntff_hook()
    res = run_bass_kernel_spmd(nc, in_maps, list(range(N_CORES)), trace=trace)
    LAST_RESULTS = res

    all_rows = np.concatenate([res.results[c]["out"] for c in range(N_CORES)],
                              axis=0)
    return np.ascontiguousarray(all_rows[rows].astype(np.float32))


# revision 15
# speedup vs baseline: 3.6067x; 1.1173x over previous
"""GCN classifier (GCNConv + LayerNorm + ReLU + Linear) on 8 Trainium2 NeuronCores.

v2 strategy (self-contained; sized for N=100000, E=1600000, 128 ch, 16 classes):
  out = LN((A @ x) @ W1 + b1).relu() @ Wfc + bfc,  A = normalized adjacency.

  Profiling insights driving this design (vs v1 baseline @ 2.48ms):
  - SWDGE descriptor generation on the Q7 cores is ~8ns/descriptor and was
    2.0ms serial on one core pair. Fix: num_swdge_queues=4, one gather call
    per source bank on its own queue_num -> 4 Q7 pairs generate in parallel.
  - DVE tensor_scalar/copy can enter 2-port perf mode which takes an
    exclusive lock on the SBUF port shared with GPSIMD -> one-hot builds
    were blocking descriptor generation (and vice versa), 4.2ms of DVE time.
    Fix: build one-hot slabs with tensor_tensor (never contends) using
    stride-0 repeat APs; PSUM evacuations / scaling moved to the ACT engine
    (own SBUF port).
  - fp32 matmuls are 4 cycles/row on the PE; fp16 is 1. Everything on the
    matmul path is fp16 (tolerance is 2e-2; fp16 keeps us ~1e-3).
  - Padding trimmed: nodes are packed into 888 tiles so every (tile, bank)
    cell fits exactly K=4 chunks of 128 edges (~7% pad vs ~30%); self-loops
    are not gathered at all - they stream as dense 128-row blocks from a
    tile-permuted fp16 copy of x via HWDGE (free of Q7 descriptor cost).
  - LayerNorm mean comes free as an extra (negated row-mean) column of the
    W1 matmul; LN affine + ReLU fold into one ACT op in transposed layout.
"""
import heapq
import os

import numpy as np

N_NODES = 100000
IN_CH = 128
HIDDEN = 128
NUM_CLASSES = 16
LN_EPS = 1e-5
N_CORES = 8
P = 128
BANK = 25000
NBANK = 4
K = 4                 # chunks per (tile, bank)
CPT = NBANK * K + 1   # chunks per tile (16 gather + 1 dense self block)
CELLCAP = K * P       # max edges per (tile, bank)
TILES = 888
TPC = TILES // N_CORES
GS = 8                # tiles per gather group

LAST_RESULTS = None
_PROGRAM_CACHE = {}


def _groups():
    out = []
    t = 0
    while t < TPC:
        s = min(GS, TPC - t)
        out.append((s, t))
        t += s
    return out


def _call_col_bases():
    """Column base (in 16-wide int16 idx columns) of each (group, bank) gather."""
    bases = []
    run = 0
    for s, _ in _groups():
        row = []
        for _b in range(NBANK):
            row.append(run)
            run += s * K * P // 16
        bases.append(row)
    return bases, run


# ----------------------------------------------------------------------------
# host-side preprocessing
# ----------------------------------------------------------------------------
def _assign_tiles(dst, eb, cnt_nb):
    """LPT-pack nodes into TILES tiles (<=128 nodes each), then repair so every
    (tile, bank) cell holds <= CELLCAP edges."""
    N = N_NODES
    cnt = cnt_nb.sum(axis=1)
    order = np.argsort(-cnt, kind="stable")
    heap = [(0, t) for t in range(TILES)]
    heapq.heapify(heap)
    node_cnt = np.zeros(TILES, dtype=np.int64)
    edge_sum = np.zeros(TILES, dtype=np.int64)
    node_tile = np.empty(N, dtype=np.int64)
    for nd in order:
        while True:
            s, t = heapq.heappop(heap)
            if node_cnt[t] < P:
                break
        node_tile[nd] = t
        node_cnt[t] += 1
        edge_sum[t] += cnt[nd]
        if node_cnt[t] < P:
            heapq.heappush(heap, (edge_sum[t], t))

    # repair per-bank overflows
    for _ in range(64):
        cell = np.zeros((TILES, NBANK), dtype=np.int64)
        np.add.at(cell, (node_tile[dst], eb), 1)
        over = np.argwhere(cell > CELLCAP)
        if len(over) == 0:
            break
        node_cnt = np.bincount(node_tile, minlength=TILES)
        for t, b in over:
            excess = cell[t, b] - CELLCAP
            if excess <= 0:
                continue
            nodes_t = np.where(node_tile == t)[0]
            cand = nodes_t[np.argsort(-cnt_nb[nodes_t, b], kind="stable")]
            for nd in cand:
                if excess <= 0:
                    break
                c_nd = cnt_nb[nd]
                if c_nd[b] == 0:
                    break
                ok = (node_cnt < P) & ((cell + c_nd[None, :]) <= CELLCAP).all(axis=1)
                ok[t] = False
                if not ok.any():
                    continue
                cand_t2 = np.where(ok)[0]
                t2 = cand_t2[np.argmin(cell[cand_t2].sum(axis=1))]
                node_tile[nd] = t2
                cell[t] -= c_nd
                cell[t2] += c_nd
                node_cnt[t] -= 1
                node_cnt[t2] += 1
                excess = cell[t, b] - CELLCAP
    else:
        raise RuntimeError("tile repair did not converge")

    # compact slots within each tile
    order2 = np.argsort(node_tile, kind="stable")
    tile_sorted = node_tile[order2]
    starts = np.zeros(TILES + 1, dtype=np.int64)
    np.cumsum(np.bincount(tile_sorted, minlength=TILES), out=starts[1:])
    node_slot = np.empty(N, dtype=np.int64)
    node_slot[order2] = np.arange(N) - starts[tile_sorted]
    assert (node_slot < P).all()
    return node_tile, node_slot


def _preprocess(edge_index, edge_weight):
    src = np.asarray(edge_index[0], dtype=np.int64)
    dst = np.asarray(edge_index[1], dtype=np.int64)
    w = np.asarray(edge_weight, dtype=np.float32)
    N = N_NODES

    deg = np.bincount(dst, weights=w.astype(np.float64), minlength=N) + 1.0
    dinv = (1.0 / np.sqrt(deg)).astype(np.float32)
    norm = (dinv[src] * w * dinv[dst]).astype(np.float32)
    selfnorm = (dinv.astype(np.float64) ** 2).astype(np.float32)  # 1/deg

    eb = src // BANK
    cnt_nb = np.zeros((N, NBANK), dtype=np.int64)
    np.add.at(cnt_nb, (dst, eb), 1)
    node_tile, node_slot = _assign_tiles(dst, eb, cnt_nb)

    # per-edge (tile, bank) cell position
    et = node_tile[dst]
    keys = et * NBANK + eb
    eorder = np.argsort(keys, kind="stable")
    keys_s = keys[eorder]
    cum = np.zeros(TILES * NBANK + 1, dtype=np.int64)
    np.cumsum(np.bincount(keys_s, minlength=TILES * NBANK), out=cum[1:])
    pos = np.arange(len(keys_s)) - cum[keys_s]
    kk = pos // P
    lane = pos % P
    assert (kk < K).all(), "cell overflow after repair"

    src_s = src[eorder]
    dst_s = dst[eorder]
    et_s = et[eorder]
    eb_s = eb[eorder]
    norm_s = norm[eorder]

    # tile-major metadata, duplicated-pair layout [128, tile*(16*2) + (bank*K+kk)*2 + {0,1}]
    # (pairs give every DVE operand an innermost stride-1 dim -> 2x perf mode)
    GCH = NBANK * K  # gathered chunks per tile (self handled via selfoh)
    MCOLS = TILES * GCH * 2
    mcol = (et_s * GCH + eb_s * K + kk) * 2
    dstl_all = np.zeros((P, MCOLS), dtype=np.float16)
    norm_all = np.zeros((P, MCOLS), dtype=np.float16)
    dstl_all[lane, mcol] = node_slot[dst_s].astype(np.float16)
    dstl_all[lane, mcol + 1] = dstl_all[lane, mcol]
    norm_all[lane, mcol] = norm_s.astype(np.float16)
    norm_all[lane, mcol + 1] = norm_all[lane, mcol]

    # precomputed self-loop one-hot diag blocks [128, TILES*128] (tile-major)
    selfoh = np.zeros((P, TILES * P), dtype=np.float16)
    selfoh[node_slot, node_tile * P + node_slot] = selfnorm.astype(np.float16)

    # gather indices, call-major: per core, per (group, bank) call,
    # within call linear i = (j*K + kk)*128 + lane
    bases, IDXC = _call_col_bases()
    core = et_s // TPC
    tl = et_s % TPC
    g = tl // GS
    j = tl % GS
    cb = np.asarray([[bases[gi][bi] for bi in range(NBANK)]
                     for gi in range(len(bases))], dtype=np.int64)
    i_lin = (j * K + kk) * P + lane
    col16 = core * IDXC + cb[g, eb_s] + i_lin // 16
    row16 = i_lin % 16
    idx16 = np.zeros((16, N_CORES * IDXC), dtype=np.int16)
    idx16[row16, col16] = (src_s % BANK).astype(np.int16)
    idx_all = np.tile(idx16, (8, 1))

    return dict(
        idx_all=idx_all, norm_all=norm_all, dstl_all=dstl_all, selfoh=selfoh,
        node_tile=node_tile, node_slot=node_slot, IDXC=IDXC,
    )


# ----------------------------------------------------------------------------
# device program
# ----------------------------------------------------------------------------
def _build_program():
    from contextlib import ExitStack
    import concourse.bass as bass
    import concourse.tile as tile
    from concourse import bacc, mybir

    f32 = mybir.dt.float32
    f16 = mybir.dt.float16
    i16 = mybir.dt.int16
    H = HIDDEN
    GCH = NBANK * K
    MC = TPC * GCH * 2
    bases, IDXC = _call_col_bases()

    nc = bacc.Bacc("TRN2", target_bir_lowering=False, debug=False,
                   num_devices=N_CORES, num_swdge_queues=4)
    xb = [nc.dram_tensor(f"xb{b}", [BANK, IN_CH], f16, kind="ExternalInput").ap()
          for b in range(NBANK)]
    xperm_d = nc.dram_tensor("xperm", [TPC * P, IN_CH], f16, kind="ExternalInput").ap()
    idx_d = nc.dram_tensor("idx", [P, IDXC], i16, kind="ExternalInput").ap()
    dstl_d = nc.dram_tensor("dstl", [P, MC], f16, kind="ExternalInput").ap()
    norm_d = nc.dram_tensor("normv", [P, MC], f16, kind="ExternalInput").ap()
    selfoh_d = nc.dram_tensor("selfoh", [P, TPC * P], f16, kind="ExternalInput").ap()
    w1_d = nc.dram_tensor("W1aug", [IN_CH, H + 1], f16, kind="ExternalInput").ap()
    wfc_d = nc.dram_tensor("Wfc", [H, NUM_CLASSES], f16, kind="ExternalInput").ap()
    c1_d = nc.dram_tensor("c1", [1, H], f32, kind="ExternalInput").ap()
    lng_d = nc.dram_tensor("ln_g", [H, 1], f32, kind="ExternalInput").ap()
    lnb_d = nc.dram_tensor("ln_b", [H, 1], f32, kind="ExternalInput").ap()
    bfc_d = nc.dram_tensor("bfc", [1, NUM_CLASSES], f32, kind="ExternalInput").ap()
    iota_d = nc.dram_tensor("iota", [1, P], f16, kind="ExternalInput").ap()
    idm_d = nc.dram_tensor("idm", [P, P], f16, kind="ExternalInput").ap()
    out_d = nc.dram_tensor("out", [TPC * P, NUM_CLASSES], f32,
                           kind="ExternalOutput").ap()

    def bcast(src_ap, parts=P):
        return bass.AP(tensor=src_ap.tensor, offset=src_ap.offset,
                       ap=[[0, parts]] + list(src_ap.ap[1:]))

    def rep_mid(ap2d, n):
        """[p, q] -> [p, n, q] with the middle dim broadcast (stride 0)."""
        a = list(ap2d.ap)
        return bass.AP(tensor=ap2d.tensor, offset=ap2d.offset,
                       ap=[a[0], [0, n], a[1]])

    # 4D APs for the one-hot slab build; every operand keeps an innermost
    # stride-1 dim of size 2 so the DVE can enter 2x_1P perf mode.
    def meta_rep4(ap2d):
        """paired meta [p, 2*GCH] -> [p, GCH, 64, 2]; value const along dim 64."""
        a = list(ap2d.ap)
        return bass.AP(tensor=ap2d.tensor, offset=ap2d.offset,
                       ap=[a[0], [2, GCH], [0, P // 2], [1, 2]])

    def iota_rep4(ap2d):
        """IOTA [p, 128] -> [p, GCH, 64, 2]; iota along the last two dims."""
        a = list(ap2d.ap)
        return bass.AP(tensor=ap2d.tensor, offset=ap2d.offset,
                       ap=[a[0], [0, GCH], [2, P // 2], [1, 2]])

    def slab4(ap2d):
        """slab [p, GCH*128] -> [p, GCH, 64, 2] contiguous."""
        a = list(ap2d.ap)
        return bass.AP(tensor=ap2d.tensor, offset=ap2d.offset,
                       ap=[a[0], [P, GCH], [2, P // 2], [1, 2]])

    AL = mybir.AluOpType
    AF = mybir.ActivationFunctionType

    with tile.TileContext(nc) as tc, ExitStack() as ctx:
        consts = ctx.enter_context(tc.tile_pool(name="consts", bufs=1))
        gpool = ctx.enter_context(tc.tile_pool(name="gather", bufs=2))
        ohp = ctx.enter_context(tc.tile_pool(name="onehot", bufs=3))
        sp = ctx.enter_context(tc.tile_pool(name="work", bufs=4))
        t1p = ctx.enter_context(tc.tile_pool(name="t1c", bufs=2 * GS))
        statp = ctx.enter_context(tc.tile_pool(name="stats", bufs=4 * GS))
        pp_ps = ctx.enter_context(tc.tile_pool(name="pp_ps", bufs=2, space="PSUM"))
        agg_ps = ctx.enter_context(tc.tile_pool(name="agg_ps", bufs=2, space="PSUM"))
        tr_ps = ctx.enter_context(tc.tile_pool(name="tr_ps", bufs=2, space="PSUM"))
        fc_ps = ctx.enter_context(tc.tile_pool(name="fc_ps", bufs=2, space="PSUM"))

        W1s = consts.tile([IN_CH, H + 1], f16)
        nc.sync.dma_start(W1s[:], w1_d[:])
        Wfcs = consts.tile([H, NUM_CLASSES], f16)
        nc.sync.dma_start(Wfcs[:], wfc_d[:])
        C1s = consts.tile([P, H], f32)
        nc.sync.dma_start(C1s[:], bcast(c1_d))
        Gcol = consts.tile([H, 1], f32)
        nc.sync.dma_start(Gcol[:], lng_d[:])
        Bcol = consts.tile([H, 1], f32)
        nc.sync.dma_start(Bcol[:], lnb_d[:])
        BFCs = consts.tile([P, NUM_CLASSES], f32)
        nc.sync.dma_start(BFCs[:], bcast(bfc_d))
        IOTA = consts.tile([P, P], f16)
        nc.sync.dma_start(IOTA[:], bcast(iota_d))
        idents = consts.tile([P, P], f16)
        nc.sync.dma_start(idents[:], idm_d[:])
        eps_t = consts.tile([P, 1], f32)
        nc.vector.memset(eps_t[:], LN_EPS)

        idx_s = consts.tile([P, IDXC], i16)
        nc.sync.dma_start(idx_s[:], idx_d[:])
        dstl_s = consts.tile([P, MC], f16)
        nc.sync.dma_start(dstl_s[:], dstl_d[:])
        norm_s = consts.tile([P, MC], f16)
        nc.sync.dma_start(norm_s[:], norm_d[:])
        selfoh_s = consts.tile([P, TPC * P], f16)
        nc.sync.dma_start(selfoh_s[:], selfoh_d[:])

        out_acc = consts.tile([P, TPC * NUM_CLASSES], f32)

        for gi, (s, tb) in enumerate(_groups()):
            Gg = gpool.tile([P, s * CPT, IN_CH], f16, tag="Gg")
            for b in range(NBANK):
                n = s * K * P
                cbase = bases[gi][b]
                nc.gpsimd.dma_gather(
                    out_ap=Gg[:, b * s * K:(b + 1) * s * K, :],
                    in_ap=xb[b][:],
                    idxs_ap=idx_s[:, cbase:cbase + n // 16],
                    num_idxs=n, num_idxs_reg=n, elem_size=IN_CH,
                    single_packet=False, queue_num=b,
                )
            # dense self blocks via HWDGE
            xp_in = bass.AP(tensor=xperm_d.tensor,
                            offset=xperm_d.offset + tb * P * IN_CH,
                            ap=[[IN_CH, P], [P * IN_CH, s], [1, IN_CH]])
            nc.sync.dma_start(Gg[:, NBANK * K * s:NBANK * K * s + s, :], xp_in)

            vars_g = statp.tile([P, s], f32, tag="vars")
            t1cs = []
            for j in range(s):
                t = tb + j
                mc0 = t * GCH * 2
                dsl = dstl_s[:, mc0:mc0 + GCH * 2]
                nsl = norm_s[:, mc0:mc0 + GCH * 2]
                tmp = ohp.tile([P, GCH * P], f16, tag="tmp")
                nc.vector.tensor_tensor(out=slab4(tmp[:]), in0=meta_rep4(dsl),
                                        in1=iota_rep4(IOTA[:]), op=AL.is_equal)
                ohs = ohp.tile([P, GCH * P], f16, tag="ohs")
                nc.vector.tensor_tensor(out=slab4(ohs[:]), in0=slab4(tmp[:]),
                                        in1=meta_rep4(nsl), op=AL.mult)

                Pp = pp_ps.tile([IN_CH, P], f32, space="PSUM")
                for c in range(CPT):
                    if c < GCH:
                        gcol = (c // K) * s * K + j * K + (c % K)
                        rhs = ohs[:, c * P:(c + 1) * P]
                    else:
                        gcol = GCH * s + j
                        rhs = selfoh_s[:, t * P:(t + 1) * P]
                    nc.tensor.matmul(Pp[:], lhsT=Gg[:, gcol, :], rhs=rhs,
                                     start=(c == 0), stop=(c == CPT - 1))
                Ps = sp.tile([IN_CH, P], f16, tag="Ps")
                nc.scalar.activation(out=Ps[:], in_=Pp[:], func=AF.Copy)
                agg = agg_ps.tile([P, H + 1], f32, space="PSUM")
                nc.tensor.matmul(agg[:], lhsT=Ps[:], rhs=W1s[:],
                                 start=True, stop=True)
                mu = statp.tile([P, 1], f32, tag="mu")
                nc.scalar.activation(out=mu[:], in_=agg[:, H:H + 1], func=AF.Copy)
                t1c = t1p.tile([P, H], f32, tag="t1c")
                nc.vector.scalar_tensor_tensor(
                    out=t1c[:], in0=agg[:, 0:H], scalar=mu[:], in1=C1s[:],
                    op0=AL.add, op1=AL.add)
                sq = sp.tile([P, H], f32, tag="sq")
                nc.scalar.activation(out=sq[:], in_=t1c[:], func=AF.Square,
                                     accum_out=vars_g[:, j:j + 1])
                t1cs.append(t1c)

            stdg = statp.tile([P, s], f32, tag="std")
            nc.scalar.activation(out=stdg[:], in_=vars_g[:], func=AF.Sqrt,
                                 bias=eps_t[:], scale=1.0 / H)
            rstd = statp.tile([P, s], f32, tag="rstd")
            nc.vector.reciprocal(out=rstd[:], in_=stdg[:])

            fc = fc_ps.tile([P, s * NUM_CLASSES], f32, space="PSUM")
            for j in range(s):
                t1n = sp.tile([P, H], f16, tag="t1n")
                nc.scalar.activation(out=t1n[:], in_=t1cs[j][:], func=AF.Copy,
                                     scale=rstd[:, j:j + 1])
                yT = tr_ps.tile([H, P], f16, space="PSUM")
                nc.tensor.transpose(out=yT[:], in_=t1n[:], identity=idents[:])
                hrT = sp.tile([H, P], f16, tag="hrT")
                nc.scalar.activation(out=hrT[:], in_=yT[:], func=AF.Relu,
                                     scale=Gcol[:], bias=Bcol[:])
                nc.tensor.matmul(fc[:, j * NUM_CLASSES:(j + 1) * NUM_CLASSES],
                                 lhsT=hrT[:], rhs=Wfcs[:], start=True, stop=True)

            oslice = out_acc[:, tb * NUM_CLASSES:(tb + s) * NUM_CLASSES]
            o3 = oslice.rearrange("p (t c) -> p t c", c=NUM_CLASSES)
            f3 = fc[:].rearrange("p (t c) -> p t c", c=NUM_CLASSES)
            nc.vector.tensor_tensor(out=o3, in0=f3,
                                    in1=rep_mid(BFCs[:], s), op=AL.add)

        out_view = out_d.rearrange("(t p) c -> p t c", p=P)
        acc_view = out_acc[:].rearrange("p (t c) -> p t c", c=NUM_CLASSES)
        nc.sync.dma_start(out_view, acc_view)

    nc.compile()
    return nc


def _ensure_ntff_hook():
    import sys, types
    try:
        from antenv.axon_hooks import get_axon_ntff_profile_hook  # noqa: F401
        return
    except ImportError:
        pass
    mod = types.ModuleType("antenv.axon_hooks")
    _hook = [None]
    mod.set_axon_ntff_profile_hook = lambda h: _hook.__setitem__(0, h)
    mod.get_axon_ntff_profile_hook = lambda: _hook[0]
    sys.modules["antenv.axon_hooks"] = mod
    try:
        import antenv
        antenv.axon_hooks = mod
    except ImportError:
        pass
    try:
        from trn_agent_boot.trn_boot import _ntff_profile_via_ctypes
        mod.set_axon_ntff_profile_hook(
            _ntff_profile_via_ctypes("/opt/axon/libaxon_pjrt.so"))
    except Exception:
        pass


# ----------------------------------------------------------------------------
# entry point
# ----------------------------------------------------------------------------
def kernel(x, edge_index, edge_weight, W1, b1, ln_g, ln_b, Wfc, bfc):
    global LAST_RESULTS
    from concourse.bass_utils import run_bass_kernel_spmd

    x16 = np.asarray(x, dtype=np.float32).astype(np.float16)
    meta = _preprocess(edge_index, edge_weight)
    IDXC = meta["IDXC"]

    if "prog" not in _PROGRAM_CACHE:
        _PROGRAM_CACHE["prog"] = _build_program()
    nc = _PROGRAM_CACHE["prog"]

    W1f = np.asarray(W1, np.float32)
    W1aug = np.zeros((IN_CH, HIDDEN + 1), dtype=np.float16)
    W1aug[:, :HIDDEN] = W1f.astype(np.float16)
    W1aug[:, HIDDEN] = (-W1f.mean(axis=1)).astype(np.float16)
    b1f = np.asarray(b1, np.float32).reshape(-1)
    c1 = (b1f - b1f.mean()).reshape(1, HIDDEN).astype(np.float32)

    xperm = np.zeros((TILES * P, IN_CH), dtype=np.float16)
    rows = meta["node_tile"] * P + meta["node_slot"]
    xperm[rows] = x16

    banks = {}
    for b in range(NBANK):
        blk = np.zeros((BANK, IN_CH), dtype=np.float16)
        seg = x16[b * BANK:(b + 1) * BANK]
        blk[:len(seg)] = seg
        banks[f"xb{b}"] = blk

    common = dict(
        banks,
        W1aug=W1aug,
        Wfc=np.asarray(Wfc, np.float32).astype(np.float16),
        c1=c1,
        ln_g=np.asarray(ln_g, np.float32).reshape(HIDDEN, 1),
        ln_b=np.asarray(ln_b, np.float32).reshape(HIDDEN, 1),
        bfc=np.asarray(bfc, np.float32).reshape(1, NUM_CLASSES),
        iota=np.arange(P, dtype=np.float16).reshape(1, P),
        idm=np.eye(P, dtype=np.float16),
    )
    MC = TPC * NBANK * K * 2
    in_maps = []
    for core in range(N_CORES):
        msl = slice(core * MC, (core + 1) * MC)
        ssl = slice(core * TPC * P, (core + 1) * TPC * P)
        in_maps.append(dict(
            common,
            idx=np.ascontiguousarray(meta["idx_all"][:, core * IDXC:(core + 1) * IDXC]),
            dstl=np.ascontiguousarray(meta["dstl_all"][:, msl]),
            normv=np.ascontiguousarray(meta["norm_all"][:, msl]),
            selfoh=np.ascontiguousarray(meta["selfoh"][:, ssl]),
            xperm=np.ascontiguousarray(xperm[ssl]),
        ))

    trace = bool(os.environ.get("KERNEL_TRACE"))
    if trace:
        _ensure_ntff_hook()
    res = run_bass_kernel_spmd(nc, in_maps, list(range(N_CORES)), trace=trace)
    LAST_RESULTS = res

    all_rows = np.concatenate([res.results[c]["out"] for c in range(N_CORES)],
                              axis=0)
    return np.ascontiguousarray(all_rows[rows].astype(np.float32))


# revision 17
# speedup vs baseline: 3.9641x; 1.0991x over previous
"""GCN classifier (GCNConv + LayerNorm + ReLU + Linear) on 8 Trainium2 NeuronCores.

v2 strategy (self-contained; sized for N=100000, E=1600000, 128 ch, 16 classes):
  out = LN((A @ x) @ W1 + b1).relu() @ Wfc + bfc,  A = normalized adjacency.

  Profiling insights driving this design (vs v1 baseline @ 2.48ms):
  - SWDGE descriptor generation on the Q7 cores is ~8ns/descriptor and was
    2.0ms serial on one core pair. Fix: num_swdge_queues=4, one gather call
    per source bank on its own queue_num -> 4 Q7 pairs generate in parallel.
  - DVE tensor_scalar/copy can enter 2-port perf mode which takes an
    exclusive lock on the SBUF port shared with GPSIMD -> one-hot builds
    were blocking descriptor generation (and vice versa), 4.2ms of DVE time.
    Fix: build one-hot slabs with tensor_tensor (never contends) using
    stride-0 repeat APs; PSUM evacuations / scaling moved to the ACT engine
    (own SBUF port).
  - fp32 matmuls are 4 cycles/row on the PE; fp16 is 1. Everything on the
    matmul path is fp16 (tolerance is 2e-2; fp16 keeps us ~1e-3).
  - Padding trimmed: nodes are packed into 888 tiles so every (tile, bank)
    cell fits exactly K=4 chunks of 128 edges (~7% pad vs ~30%); self-loops
    are not gathered at all - they stream as dense 128-row blocks from a
    tile-permuted fp16 copy of x via HWDGE (free of Q7 descriptor cost).
  - LayerNorm mean comes free as an extra (negated row-mean) column of the
    W1 matmul; LN affine + ReLU fold into one ACT op in transposed layout.
"""
import heapq
import os

import numpy as np

N_NODES = 100000
IN_CH = 128
HIDDEN = 128
NUM_CLASSES = 16
LN_EPS = 1e-5
N_CORES = 8
P = 128
BANK = 25000
NBANK = 4
K = 4                 # chunks per (tile, bank)
CPT = NBANK * K + 1   # chunks per tile (16 gather + 1 dense self block)
CELLCAP = K * P       # max edges per (tile, bank)
TILES = 848
TPC = TILES // N_CORES
GS = 8                # tiles per gather group

LAST_RESULTS = None
_PROGRAM_CACHE = {}


def _groups():
    out = []
    t = 0
    while t < TPC:
        s = min(GS, TPC - t)
        out.append((s, t))
        t += s
    return out


def _call_col_bases():
    """Column base (in 16-wide int16 idx columns) of each (group, bank) gather."""
    bases = []
    run = 0
    for s, _ in _groups():
        row = []
        for _b in range(NBANK):
            row.append(run)
            run += s * K * P // 16
        bases.append(row)
    return bases, run


# ----------------------------------------------------------------------------
# host-side preprocessing
# ----------------------------------------------------------------------------
def _assign_tiles(dst, eb, cnt_nb):
    """LPT-pack nodes into TILES tiles (<=128 nodes each), then repair so every
    (tile, bank) cell holds <= CELLCAP edges."""
    N = N_NODES
    cnt = cnt_nb.sum(axis=1)
    order = np.argsort(-cnt, kind="stable")
    heap = [(0, t) for t in range(TILES)]
    heapq.heapify(heap)
    node_cnt = np.zeros(TILES, dtype=np.int64)
    edge_sum = np.zeros(TILES, dtype=np.int64)
    node_tile = np.empty(N, dtype=np.int64)
    for nd in order:
        while True:
            s, t = heapq.heappop(heap)
            if node_cnt[t] < P:
                break
        node_tile[nd] = t
        node_cnt[t] += 1
        edge_sum[t] += cnt[nd]
        if node_cnt[t] < P:
            heapq.heappush(heap, (edge_sum[t], t))

    # repair per-bank overflows
    for _ in range(64):
        cell = np.zeros((TILES, NBANK), dtype=np.int64)
        np.add.at(cell, (node_tile[dst], eb), 1)
        over = np.argwhere(cell > CELLCAP)
        if len(over) == 0:
            break
        node_cnt = np.bincount(node_tile, minlength=TILES)
        for t, b in over:
            excess = cell[t, b] - CELLCAP
            if excess <= 0:
                continue
            nodes_t = np.where(node_tile == t)[0]
            cand = nodes_t[np.argsort(-cnt_nb[nodes_t, b], kind="stable")]
            for nd in cand:
                if excess <= 0:
                    break
                c_nd = cnt_nb[nd]
                if c_nd[b] == 0:
                    break
                ok = (node_cnt < P) & ((cell + c_nd[None, :]) <= CELLCAP).all(axis=1)
                ok[t] = False
                if not ok.any():
                    continue
                cand_t2 = np.where(ok)[0]
                t2 = cand_t2[np.argmin(cell[cand_t2].sum(axis=1))]
                node_tile[nd] = t2
                cell[t] -= c_nd
                cell[t2] += c_nd
                node_cnt[t] -= 1
                node_cnt[t2] += 1
                excess = cell[t, b] - CELLCAP
    else:
        raise RuntimeError("tile repair did not converge")

    # compact slots within each tile
    order2 = np.argsort(node_tile, kind="stable")
    tile_sorted = node_tile[order2]
    starts = np.zeros(TILES + 1, dtype=np.int64)
    np.cumsum(np.bincount(tile_sorted, minlength=TILES), out=starts[1:])
    node_slot = np.empty(N, dtype=np.int64)
    node_slot[order2] = np.arange(N) - starts[tile_sorted]
    assert (node_slot < P).all()
    return node_tile, node_slot


def _preprocess(edge_index, edge_weight):
    src = np.asarray(edge_index[0], dtype=np.int64)
    dst = np.asarray(edge_index[1], dtype=np.int64)
    w = np.asarray(edge_weight, dtype=np.float32)
    N = N_NODES

    deg = np.bincount(dst, weights=w.astype(np.float64), minlength=N) + 1.0
    dinv = (1.0 / np.sqrt(deg)).astype(np.float32)
    norm = (dinv[src] * w * dinv[dst]).astype(np.float32)
    selfnorm = (dinv.astype(np.float64) ** 2).astype(np.float32)  # 1/deg

    eb = src // BANK
    cnt_nb = np.zeros((N, NBANK), dtype=np.int64)
    np.add.at(cnt_nb, (dst, eb), 1)
    node_tile, node_slot = _assign_tiles(dst, eb, cnt_nb)

    # per-edge (tile, bank) cell position
    et = node_tile[dst]
    keys = et * NBANK + eb
    eorder = np.argsort(keys, kind="stable")
    keys_s = keys[eorder]
    cum = np.zeros(TILES * NBANK + 1, dtype=np.int64)
    np.cumsum(np.bincount(keys_s, minlength=TILES * NBANK), out=cum[1:])
    pos = np.arange(len(keys_s)) - cum[keys_s]
    kk = pos // P
    lane = pos % P
    assert (kk < K).all(), "cell overflow after repair"

    src_s = src[eorder]
    dst_s = dst[eorder]
    et_s = et[eorder]
    eb_s = eb[eorder]
    norm_s = norm[eorder]

    # tile-major metadata, duplicated-pair layout [128, tile*(16*2) + (bank*K+kk)*2 + {0,1}]
    # (pairs give every DVE operand an innermost stride-1 dim -> 2x perf mode)
    GCH = NBANK * K  # gathered chunks per tile (self handled via selfoh)
    MCOLS = TILES * GCH * 2
    mcol = (et_s * GCH + eb_s * K + kk) * 2
    dstl_all = np.zeros((P, MCOLS), dtype=np.float16)
    norm_all = np.zeros((P, MCOLS), dtype=np.float16)
    dstl_all[lane, mcol] = node_slot[dst_s].astype(np.float16)
    dstl_all[lane, mcol + 1] = dstl_all[lane, mcol]
    norm_all[lane, mcol] = norm_s.astype(np.float16)
    norm_all[lane, mcol + 1] = norm_all[lane, mcol]

    # precomputed self-loop one-hot diag blocks [128, TILES*128] (tile-major)
    selfoh = np.zeros((P, TILES * P), dtype=np.float16)
    selfoh[node_slot, node_tile * P + node_slot] = selfnorm.astype(np.float16)

    # gather indices, call-major: per core, per (group, bank) call,
    # within call linear i = (j*K + kk)*128 + lane
    bases, IDXC = _call_col_bases()
    core = et_s // TPC
    tl = et_s % TPC
    g = tl // GS
    j = tl % GS
    cb = np.asarray([[bases[gi][bi] for bi in range(NBANK)]
                     for gi in range(len(bases))], dtype=np.int64)
    i_lin = (j * K + kk) * P + lane
    col16 = core * IDXC + cb[g, eb_s] + i_lin // 16
    row16 = i_lin % 16
    idx16 = np.zeros((16, N_CORES * IDXC), dtype=np.int16)
    idx16[row16, col16] = (src_s % BANK).astype(np.int16)
    idx_all = np.tile(idx16, (8, 1))

    return dict(
        idx_all=idx_all, norm_all=norm_all, dstl_all=dstl_all, selfoh=selfoh,
        node_tile=node_tile, node_slot=node_slot, IDXC=IDXC,
    )


# ----------------------------------------------------------------------------
# device program
# ----------------------------------------------------------------------------
def _build_program():
    from contextlib import ExitStack
    import concourse.bass as bass
    import concourse.tile as tile
    from concourse import bacc, mybir

    f32 = mybir.dt.float32
    f16 = mybir.dt.float16
    i16 = mybir.dt.int16
    H = HIDDEN
    GCH = NBANK * K
    MC = TPC * GCH * 2
    bases, IDXC = _call_col_bases()

    nc = bacc.Bacc("TRN2", target_bir_lowering=False, debug=False,
                   num_devices=N_CORES, num_swdge_queues=4)
    xb = [nc.dram_tensor(f"xb{b}", [BANK, IN_CH], f16, kind="ExternalInput").ap()
          for b in range(NBANK)]
    xperm_d = nc.dram_tensor("xperm", [TPC * P, IN_CH], f16, kind="ExternalInput").ap()
    idx_d = nc.dram_tensor("idx", [P, IDXC], i16, kind="ExternalInput").ap()
    dstl_d = nc.dram_tensor("dstl", [P, MC], f16, kind="ExternalInput").ap()
    norm_d = nc.dram_tensor("normv", [P, MC], f16, kind="ExternalInput").ap()
    selfoh_d = nc.dram_tensor("selfoh", [P, TPC * P], f16, kind="ExternalInput").ap()
    w1_d = nc.dram_tensor("W1aug", [IN_CH, H + 1], f16, kind="ExternalInput").ap()
    wfc_d = nc.dram_tensor("Wfc", [H, NUM_CLASSES], f16, kind="ExternalInput").ap()
    c1_d = nc.dram_tensor("c1", [1, H], f32, kind="ExternalInput").ap()
    lng_d = nc.dram_tensor("ln_g", [H, 1], f32, kind="ExternalInput").ap()
    lnb_d = nc.dram_tensor("ln_b", [H, 1], f32, kind="ExternalInput").ap()
    bfc_d = nc.dram_tensor("bfc", [1, NUM_CLASSES], f32, kind="ExternalInput").ap()
    iota_d = nc.dram_tensor("iota", [1, P], f16, kind="ExternalInput").ap()
    idm_d = nc.dram_tensor("idm", [P, P], f16, kind="ExternalInput").ap()
    out_d = nc.dram_tensor("out", [TPC * P, NUM_CLASSES], f32,
                           kind="ExternalOutput").ap()

    def bcast(src_ap, parts=P):
        return bass.AP(tensor=src_ap.tensor, offset=src_ap.offset,
                       ap=[[0, parts]] + list(src_ap.ap[1:]))

    def rep_mid(ap2d, n):
        """[p, q] -> [p, n, q] with the middle dim broadcast (stride 0)."""
        a = list(ap2d.ap)
        return bass.AP(tensor=ap2d.tensor, offset=ap2d.offset,
                       ap=[a[0], [0, n], a[1]])

    # 4D APs for the one-hot slab build; every operand keeps an innermost
    # stride-1 dim of size 2 so the DVE can enter 2x_1P perf mode.
    def meta_rep4(ap2d):
        """paired meta [p, 2*GCH] -> [p, GCH, 64, 2]; value const along dim 64."""
        a = list(ap2d.ap)
        return bass.AP(tensor=ap2d.tensor, offset=ap2d.offset,
                       ap=[a[0], [2, GCH], [0, P // 2], [1, 2]])

    def iota_rep4(ap2d):
        """IOTA [p, 128] -> [p, GCH, 64, 2]; iota along the last two dims."""
        a = list(ap2d.ap)
        return bass.AP(tensor=ap2d.tensor, offset=ap2d.offset,
                       ap=[a[0], [0, GCH], [2, P // 2], [1, 2]])

    def slab4(ap2d):
        """slab [p, GCH*128] -> [p, GCH, 64, 2] contiguous."""
        a = list(ap2d.ap)
        return bass.AP(tensor=ap2d.tensor, offset=ap2d.offset,
                       ap=[a[0], [P, GCH], [2, P // 2], [1, 2]])

    AL = mybir.AluOpType
    AF = mybir.ActivationFunctionType

    with tile.TileContext(nc) as tc, ExitStack() as ctx:
        consts = ctx.enter_context(tc.tile_pool(name="consts", bufs=1))
        gpool = ctx.enter_context(tc.tile_pool(name="gather", bufs=2))
        ohp = ctx.enter_context(tc.tile_pool(name="onehot", bufs=3))
        sp = ctx.enter_context(tc.tile_pool(name="work", bufs=4))
        t1p = ctx.enter_context(tc.tile_pool(name="t1c", bufs=2 * GS))
        statp = ctx.enter_context(tc.tile_pool(name="stats", bufs=4 * GS))
        pp_ps = ctx.enter_context(tc.tile_pool(name="pp_ps", bufs=2, space="PSUM"))
        agg_ps = ctx.enter_context(tc.tile_pool(name="agg_ps", bufs=2, space="PSUM"))
        tr_ps = ctx.enter_context(tc.tile_pool(name="tr_ps", bufs=2, space="PSUM"))
        fc_ps = ctx.enter_context(tc.tile_pool(name="fc_ps", bufs=2, space="PSUM"))

        W1s = consts.tile([IN_CH, H + 1], f16)
        nc.sync.dma_start(W1s[:], w1_d[:])
        Wfcs = consts.tile([H, NUM_CLASSES], f16)
        nc.sync.dma_start(Wfcs[:], wfc_d[:])
        C1s = consts.tile([P, H], f32)
        nc.sync.dma_start(C1s[:], bcast(c1_d))
        Gcol = consts.tile([H, 1], f32)
        nc.sync.dma_start(Gcol[:], lng_d[:])
        Bcol = consts.tile([H, 1], f32)
        nc.sync.dma_start(Bcol[:], lnb_d[:])
        BFCs = consts.tile([P, NUM_CLASSES], f32)
        nc.sync.dma_start(BFCs[:], bcast(bfc_d))
        IOTA = consts.tile([P, P], f16)
        nc.sync.dma_start(IOTA[:], bcast(iota_d))
        idents = consts.tile([P, P], f16)
        nc.sync.dma_start(idents[:], idm_d[:])
        eps_t = consts.tile([P, 1], f32)
        nc.vector.memset(eps_t[:], LN_EPS)

        idx_s = consts.tile([P, IDXC], i16)
        nc.sync.dma_start(idx_s[:], idx_d[:])
        dstl_s = consts.tile([P, MC], f16)
        nc.sync.dma_start(dstl_s[:], dstl_d[:])
        norm_s = consts.tile([P, MC], f16)
        nc.sync.dma_start(norm_s[:], norm_d[:])
        selfoh_s = consts.tile([P, TPC * P], f16)
        nc.sync.dma_start(selfoh_s[:], selfoh_d[:])

        out_acc = consts.tile([P, TPC * NUM_CLASSES], f32)

        # Software-pipelined emission: the PE stream is kept dense by skewing
        # every cross-engine round trip behind enough chunk-matmul work that
        # its dependencies are already resolved when the PE (or DVE/ACT FIFO
        # head) reaches it.
        def emit_agg(st):
            agg = agg_ps.tile([P, H + 1], f32, space="PSUM")
            nc.tensor.matmul(agg[:], lhsT=st["Ps"][:], rhs=W1s[:],
                             start=True, stop=True)
            st["agg"] = agg

        def emit_stats(st):
            agg = st["agg"]
            mu = statp.tile([P, 1], f32, tag="mu")
            nc.scalar.activation(out=mu[:], in_=agg[:, H:H + 1], func=AF.Copy)
            t1c = t1p.tile([P, H], f32, tag="t1c")
            nc.vector.scalar_tensor_tensor(
                out=t1c[:], in0=agg[:, 0:H], scalar=mu[:], in1=C1s[:],
                op0=AL.add, op1=AL.add)
            sq = sp.tile([P, H], f32, tag="sq")
            nc.scalar.activation(out=sq[:], in_=t1c[:], func=AF.Square,
                                 accum_out=st["vars"][:, st["j"]:st["j"] + 1])
            st["t1c"] = t1c

        def emit_b1(st):
            t1n = sp.tile([P, H], f16, tag="t1n")
            nc.scalar.activation(out=t1n[:], in_=st["t1c"][:], func=AF.Copy,
                                 scale=st["rstd"][:, st["j"]:st["j"] + 1])
            yT = tr_ps.tile([H, P], f16, space="PSUM")
            nc.tensor.transpose(out=yT[:], in_=t1n[:], identity=idents[:])
            st["yT"] = yT

        def emit_b2(st):
            hrT = sp.tile([H, P], f16, tag="hrT")
            nc.scalar.activation(out=hrT[:], in_=st["yT"][:], func=AF.Relu,
                                 scale=Gcol[:], bias=Bcol[:])
            j = st["j"]
            nc.tensor.matmul(st["fc"][:, j * NUM_CLASSES:(j + 1) * NUM_CLASSES],
                             lhsT=hrT[:], rhs=Wfcs[:], start=True, stop=True)

        def emit_outadd(gst):
            s, tb, fc = gst["s"], gst["tb"], gst["fc"]
            oslice = out_acc[:, tb * NUM_CLASSES:(tb + s) * NUM_CLASSES]
            o3 = oslice.rearrange("p (t c) -> p t c", c=NUM_CLASSES)
            f3 = fc[:, 0:s * NUM_CLASSES].rearrange("p (t c) -> p t c",
                                                    c=NUM_CLASSES)
            nc.vector.tensor_tensor(out=o3, in0=f3,
                                    in1=rep_mid(BFCs[:], s), op=AL.add)

        prev_states = None   # tile states of the previous group (pass B pending)
        prev_gst = None
        for gi, (s, tb) in enumerate(_groups()):
            Gg = gpool.tile([P, s * CPT, IN_CH], f16, tag="Gg")
            for b in range(NBANK):
                n = s * K * P
                cbase = bases[gi][b]
                nc.gpsimd.dma_gather(
                    out_ap=Gg[:, b * s * K:(b + 1) * s * K, :],
                    in_ap=xb[b][:],
                    idxs_ap=idx_s[:, cbase:cbase + n // 16],
                    num_idxs=n, num_idxs_reg=n, elem_size=IN_CH,
                    single_packet=False, queue_num=b,
                )
            # dense self blocks via HWDGE
            xp_in = bass.AP(tensor=xperm_d.tensor,
                            offset=xperm_d.offset + tb * P * IN_CH,
                            ap=[[IN_CH, P], [P * IN_CH, s], [1, IN_CH]])
            nc.sync.dma_start(Gg[:, GCH * s:GCH * s + s, :], xp_in)

            vars_g = statp.tile([P, s], f32, tag="vars")
            states = []
            for j in range(s):
                t = tb + j
                mc0 = t * GCH * 2
                dsl = dstl_s[:, mc0:mc0 + GCH * 2]
                nsl = norm_s[:, mc0:mc0 + GCH * 2]
                tmp = ohp.tile([P, GCH * P], f16, tag="tmp")
                nc.vector.tensor_tensor(out=slab4(tmp[:]), in0=meta_rep4(dsl),
                                        in1=iota_rep4(IOTA[:]), op=AL.is_equal)
                ohs = ohp.tile([P, GCH * P], f16, tag="ohs")
                nc.vector.tensor_tensor(out=slab4(ohs[:]), in0=slab4(tmp[:]),
                                        in1=meta_rep4(nsl), op=AL.mult)

                Pp = pp_ps.tile([IN_CH, P], f32, space="PSUM")
                for c in range(CPT):
                    if c < GCH:
                        gcol = (c // K) * s * K + j * K + (c % K)
                        rhs = ohs[:, c * P:(c + 1) * P]
                    else:
                        gcol = GCH * s + j
                        rhs = selfoh_s[:, t * P:(t + 1) * P]
                    nc.tensor.matmul(Pp[:], lhsT=Gg[:, gcol, :], rhs=rhs,
                                     start=(c == 0), stop=(c == CPT - 1))
                Ps = sp.tile([IN_CH, P], f16, tag="Ps")
                nc.scalar.activation(out=Ps[:], in_=Pp[:], func=AF.Copy)
                states.append({"j": j, "Ps": Ps, "vars": vars_g})

                # skewed tails: agg one tile behind, stats two tiles behind
                if j >= 1:
                    emit_agg(states[j - 1])
                if j >= 2:
                    emit_stats(states[j - 2])
                # previous group's pass B: b1 at tile j, b2 one tile later
                if prev_states is not None:
                    if j < len(prev_states):
                        emit_b1(prev_states[j])
                    if 1 <= j <= len(prev_states):
                        emit_b2(prev_states[j - 1])

            # group-boundary flush of pass A
            emit_agg(states[s - 1])
            for jj in range(max(0, s - 2), s):
                emit_stats(states[jj])
            if prev_states is not None:
                for k in range(s, len(prev_states)):
                    emit_b1(prev_states[k])
                for k in range(max(1, s), len(prev_states) + 1):
                    emit_b2(prev_states[k - 1])
                emit_outadd(prev_gst)

            stdg = statp.tile([P, s], f32, tag="std")
            nc.scalar.activation(out=stdg[:], in_=vars_g[:], func=AF.Sqrt,
                                 bias=eps_t[:], scale=1.0 / H)
            rstd = statp.tile([P, s], f32, tag="rstd")
            nc.vector.reciprocal(out=rstd[:], in_=stdg[:])
            fc = fc_ps.tile([P, s * NUM_CLASSES], f32, space="PSUM")
            for st in states:
                st["rstd"] = rstd
                st["fc"] = fc
            prev_states = states
            prev_gst = {"s": s, "tb": tb, "fc": fc}

        # final group's pass B
        for st in prev_states:
            emit_b1(st)
            emit_b2(st)
        emit_outadd(prev_gst)

        out_view = out_d.rearrange("(t p) c -> p t c", p=P)
        acc_view = out_acc[:].rearrange("p (t c) -> p t c", c=NUM_CLASSES)
        nc.sync.dma_start(out_view, acc_view)

    nc.compile()
    return nc


def _ensure_ntff_hook():
    import sys, types
    try:
        from antenv.axon_hooks import get_axon_ntff_profile_hook  # noqa: F401
        return
    except ImportError:
        pass
    mod = types.ModuleType("antenv.axon_hooks")
    _hook = [None]
    mod.set_axon_ntff_profile_hook = lambda h: _hook.__setitem__(0, h)
    mod.get_axon_ntff_profile_hook = lambda: _hook[0]
    sys.modules["antenv.axon_hooks"] = mod
    try:
        import antenv
        antenv.axon_hooks = mod
    except ImportError:
        pass
    try:
        from trn_agent_boot.trn_boot import _ntff_profile_via_ctypes
        mod.set_axon_ntff_profile_hook(
            _ntff_profile_via_ctypes("/opt/axon/libaxon_pjrt.so"))
    except Exception:
        pass


# ----------------------------------------------------------------------------
# entry point
# ----------------------------------------------------------------------------
def kernel(x, edge_index, edge_weight, W1, b1, ln_g, ln_b, Wfc, bfc):
    global LAST_RESULTS
    from concourse.bass_utils import run_bass_kernel_spmd

    x16 = np.asarray(x, dtype=np.float32).astype(np.float16)
    meta = _preprocess(edge_index, edge_weight)
    IDXC = meta["IDXC"]

    if "prog" not in _PROGRAM_CACHE:
        _PROGRAM_CACHE["prog"] = _build_program()
    nc = _PROGRAM_CACHE["prog"]

    W1f = np.asarray(W1, np.float32)
    W1aug = np.zeros((IN_CH, HIDDEN + 1), dtype=np.float16)
    W1aug[:, :HIDDEN] = W1f.astype(np.float16)
    W1aug[:, HIDDEN] = (-W1f.mean(axis=1)).astype(np.float16)
    b1f = np.asarray(b1, np.float32).reshape(-1)
    c1 = (b1f - b1f.mean()).reshape(1, HIDDEN).astype(np.float32)

    xperm = np.zeros((TILES * P, IN_CH), dtype=np.float16)
    rows = meta["node_tile"] * P + meta["node_slot"]
    xperm[rows] = x16

    banks = {}
    for b in range(NBANK):
        blk = np.zeros((BANK, IN_CH), dtype=np.float16)
        seg = x16[b * BANK:(b + 1) * BANK]
        blk[:len(seg)] = seg
        banks[f"xb{b}"] = blk

    common = dict(
        banks,
        W1aug=W1aug,
        Wfc=np.asarray(Wfc, np.float32).astype(np.float16),
        c1=c1,
        ln_g=np.asarray(ln_g, np.float32).reshape(HIDDEN, 1),
        ln_b=np.asarray(ln_b, np.float32).reshape(HIDDEN, 1),
        bfc=np.asarray(bfc, np.float32).reshape(1, NUM_CLASSES),
        iota=np.arange(P, dtype=np.float16).reshape(1, P),
        idm=np.eye(P, dtype=np.float16),
    )
    MC = TPC * NBANK * K * 2
    in_maps = []
    for core in range(N_CORES):
        msl = slice(core * MC, (core + 1) * MC)
        ssl = slice(core * TPC * P, (core + 1) * TPC * P)
        in_maps.append(dict(
            common,
            idx=np.ascontiguousarray(meta["idx_all"][:, core * IDXC:(core + 1) * IDXC]),
            dstl=np.ascontiguousarray(meta["dstl_all"][:, msl]),
            normv=np.ascontiguousarray(meta["norm_all"][:, msl]),
            selfoh=np.ascontiguousarray(meta["selfoh"][:, ssl]),
            xperm=np.ascontiguousarray(xperm[ssl]),
        ))

    trace = bool(os.environ.get("KERNEL_TRACE"))
    if trace:
        _ensure_ntff_hook()
    res = run_bass_kernel_spmd(nc, in_maps, list(range(N_CORES)), trace=trace)
    LAST_RESULTS = res

    all_rows = np.concatenate([res.results[c]["out"] for c in range(N_CORES)],
                              axis=0)
    return np.ascontiguousarray(all_rows[rows].astype(np.float32))


# revision 19
# speedup vs baseline: 4.0779x; 1.0287x over previous
"""GCN classifier (GCNConv + LayerNorm + ReLU + Linear) on 8 Trainium2 NeuronCores.

v2 strategy (self-contained; sized for N=100000, E=1600000, 128 ch, 16 classes):
  out = LN((A @ x) @ W1 + b1).relu() @ Wfc + bfc,  A = normalized adjacency.

  Profiling insights driving this design (vs v1 baseline @ 2.48ms):
  - SWDGE descriptor generation on the Q7 cores is ~8ns/descriptor and was
    2.0ms serial on one core pair. Fix: num_swdge_queues=4, one gather call
    per source bank on its own queue_num -> 4 Q7 pairs generate in parallel.
  - DVE tensor_scalar/copy can enter 2-port perf mode which takes an
    exclusive lock on the SBUF port shared with GPSIMD -> one-hot builds
    were blocking descriptor generation (and vice versa), 4.2ms of DVE time.
    Fix: build one-hot slabs with tensor_tensor (never contends) using
    stride-0 repeat APs; PSUM evacuations / scaling moved to the ACT engine
    (own SBUF port).
  - fp32 matmuls are 4 cycles/row on the PE; fp16 is 1. Everything on the
    matmul path is fp16 (tolerance is 2e-2; fp16 keeps us ~1e-3).
  - Padding trimmed: nodes are packed into 888 tiles so every (tile, bank)
    cell fits exactly K=4 chunks of 128 edges (~7% pad vs ~30%); self-loops
    are not gathered at all - they stream as dense 128-row blocks from a
    tile-permuted fp16 copy of x via HWDGE (free of Q7 descriptor cost).
  - LayerNorm mean comes free as an extra (negated row-mean) column of the
    W1 matmul; LN affine + ReLU fold into one ACT op in transposed layout.
"""
import heapq
import os

import numpy as np

N_NODES = 100000
IN_CH = 128
HIDDEN = 128
NUM_CLASSES = 16
LN_EPS = 1e-5
N_CORES = 8
P = 128
BANK = 25000
NBANK = 4
K = 4                 # chunks per (tile, bank)
CPT = NBANK * K + 1   # chunks per tile (16 gather + 1 dense self block)
CELLCAP = K * P       # max edges per (tile, bank)
TILES = 848
TPC = TILES // N_CORES
GS = 8                # tiles per gather group

LAST_RESULTS = None
_PROGRAM_CACHE = {}


def _groups():
    out = []
    t = 0
    while t < TPC:
        s = min(GS, TPC - t)
        out.append((s, t))
        t += s
    return out


def _call_col_bases():
    """Column base (in 16-wide int16 idx columns) of each (group, bank) gather."""
    bases = []
    run = 0
    for s, _ in _groups():
        row = []
        for _b in range(NBANK):
            row.append(run)
            run += s * K * P // 16
        bases.append(row)
    return bases, run


# ----------------------------------------------------------------------------
# host-side preprocessing
# ----------------------------------------------------------------------------
def _assign_tiles(dst, eb, cnt_nb):
    """LPT-pack nodes into TILES tiles (<=128 nodes each), then repair so every
    (tile, bank) cell holds <= CELLCAP edges."""
    N = N_NODES
    cnt = cnt_nb.sum(axis=1)
    order = np.argsort(-cnt, kind="stable")
    heap = [(0, t) for t in range(TILES)]
    heapq.heapify(heap)
    node_cnt = np.zeros(TILES, dtype=np.int64)
    edge_sum = np.zeros(TILES, dtype=np.int64)
    node_tile = np.empty(N, dtype=np.int64)
    for nd in order:
        while True:
            s, t = heapq.heappop(heap)
            if node_cnt[t] < P:
                break
        node_tile[nd] = t
        node_cnt[t] += 1
        edge_sum[t] += cnt[nd]
        if node_cnt[t] < P:
            heapq.heappush(heap, (edge_sum[t], t))

    # repair per-bank overflows
    for _ in range(64):
        cell = np.zeros((TILES, NBANK), dtype=np.int64)
        np.add.at(cell, (node_tile[dst], eb), 1)
        over = np.argwhere(cell > CELLCAP)
        if len(over) == 0:
            break
        node_cnt = np.bincount(node_tile, minlength=TILES)
        for t, b in over:
            excess = cell[t, b] - CELLCAP
            if excess <= 0:
                continue
            nodes_t = np.where(node_tile == t)[0]
            cand = nodes_t[np.argsort(-cnt_nb[nodes_t, b], kind="stable")]
            for nd in cand:
                if excess <= 0:
                    break
                c_nd = cnt_nb[nd]
                if c_nd[b] == 0:
                    break
                ok = (node_cnt < P) & ((cell + c_nd[None, :]) <= CELLCAP).all(axis=1)
                ok[t] = False
                if not ok.any():
                    continue
                cand_t2 = np.where(ok)[0]
                t2 = cand_t2[np.argmin(cell[cand_t2].sum(axis=1))]
                node_tile[nd] = t2
                cell[t] -= c_nd
                cell[t2] += c_nd
                node_cnt[t] -= 1
                node_cnt[t2] += 1
                excess = cell[t, b] - CELLCAP
    else:
        raise RuntimeError("tile repair did not converge")

    # compact slots within each tile
    order2 = np.argsort(node_tile, kind="stable")
    tile_sorted = node_tile[order2]
    starts = np.zeros(TILES + 1, dtype=np.int64)
    np.cumsum(np.bincount(tile_sorted, minlength=TILES), out=starts[1:])
    node_slot = np.empty(N, dtype=np.int64)
    node_slot[order2] = np.arange(N) - starts[tile_sorted]
    assert (node_slot < P).all()
    return node_tile, node_slot


def _preprocess(edge_index, edge_weight):
    src = np.asarray(edge_index[0], dtype=np.int64)
    dst = np.asarray(edge_index[1], dtype=np.int64)
    w = np.asarray(edge_weight, dtype=np.float32)
    N = N_NODES

    deg = np.bincount(dst, weights=w.astype(np.float64), minlength=N) + 1.0
    dinv = (1.0 / np.sqrt(deg)).astype(np.float32)
    norm = (dinv[src] * w * dinv[dst]).astype(np.float32)
    selfnorm = (dinv.astype(np.float64) ** 2).astype(np.float32)  # 1/deg

    eb = src // BANK
    cnt_nb = np.zeros((N, NBANK), dtype=np.int64)
    np.add.at(cnt_nb, (dst, eb), 1)
    node_tile, node_slot = _assign_tiles(dst, eb, cnt_nb)

    # per-edge (tile, bank) cell position
    et = node_tile[dst]
    keys = et * NBANK + eb
    eorder = np.argsort(keys, kind="stable")
    keys_s = keys[eorder]
    cum = np.zeros(TILES * NBANK + 1, dtype=np.int64)
    np.cumsum(np.bincount(keys_s, minlength=TILES * NBANK), out=cum[1:])
    pos = np.arange(len(keys_s)) - cum[keys_s]
    kk = pos // P
    lane = pos % P
    assert (kk < K).all(), "cell overflow after repair"

    src_s = src[eorder]
    dst_s = dst[eorder]
    et_s = et[eorder]
    eb_s = eb[eorder]
    norm_s = norm[eorder]

    # tile-major metadata, duplicated-pair layout [128, tile*(16*2) + (bank*K+kk)*2 + {0,1}]
    # (pairs give every DVE operand an innermost stride-1 dim -> 2x perf mode)
    GCH = NBANK * K  # gathered chunks per tile (self handled via selfoh)
    MCOLS = TILES * GCH * 2
    mcol = (et_s * GCH + eb_s * K + kk) * 2
    dstl_all = np.zeros((P, MCOLS), dtype=np.float16)
    norm_all = np.zeros((P, MCOLS), dtype=np.float16)
    dstl_all[lane, mcol] = node_slot[dst_s].astype(np.float16)
    dstl_all[lane, mcol + 1] = dstl_all[lane, mcol]
    norm_all[lane, mcol] = norm_s.astype(np.float16)
    norm_all[lane, mcol + 1] = norm_all[lane, mcol]

    # precomputed self-loop one-hot diag blocks [128, TILES*128] (tile-major)
    selfoh = np.zeros((P, TILES * P), dtype=np.float16)
    selfoh[node_slot, node_tile * P + node_slot] = selfnorm.astype(np.float16)

    # gather indices, call-major: per core, per (group, bank) call,
    # within call linear i = (j*K + kk)*128 + lane
    bases, IDXC = _call_col_bases()
    core = et_s // TPC
    tl = et_s % TPC
    g = tl // GS
    j = tl % GS
    cb = np.asarray([[bases[gi][bi] for bi in range(NBANK)]
                     for gi in range(len(bases))], dtype=np.int64)
    i_lin = (j * K + kk) * P + lane
    col16 = core * IDXC + cb[g, eb_s] + i_lin // 16
    row16 = i_lin % 16
    idx16 = np.zeros((16, N_CORES * IDXC), dtype=np.int16)
    idx16[row16, col16] = (src_s % BANK).astype(np.int16)
    idx_all = np.tile(idx16, (8, 1))

    return dict(
        idx_all=idx_all, norm_all=norm_all, dstl_all=dstl_all, selfoh=selfoh,
        node_tile=node_tile, node_slot=node_slot, IDXC=IDXC,
    )


# ----------------------------------------------------------------------------
# device program
# ----------------------------------------------------------------------------
def _build_program():
    from contextlib import ExitStack
    import concourse.bass as bass
    import concourse.tile as tile
    from concourse import bacc, mybir

    f32 = mybir.dt.float32
    f16 = mybir.dt.float16
    i16 = mybir.dt.int16
    H = HIDDEN
    GCH = NBANK * K
    MC = TPC * GCH * 2
    bases, IDXC = _call_col_bases()

    nc = bacc.Bacc("TRN2", target_bir_lowering=False, debug=False,
                   num_devices=N_CORES, num_swdge_queues=4)
    xb = [nc.dram_tensor(f"xb{b}", [BANK, IN_CH], f16, kind="ExternalInput").ap()
          for b in range(NBANK)]
    xperm_d = nc.dram_tensor("xperm", [TPC * P, IN_CH], f16, kind="ExternalInput").ap()
    idx_d = nc.dram_tensor("idx", [P, IDXC], i16, kind="ExternalInput").ap()
    dstl_d = nc.dram_tensor("dstl", [P, MC], f16, kind="ExternalInput").ap()
    norm_d = nc.dram_tensor("normv", [P, MC], f16, kind="ExternalInput").ap()
    selfoh_d = nc.dram_tensor("selfoh", [P, TPC * P], f16, kind="ExternalInput").ap()
    w1_d = nc.dram_tensor("W1aug", [IN_CH, H + 1], f16, kind="ExternalInput").ap()
    wfc_d = nc.dram_tensor("Wfc", [H, NUM_CLASSES], f16, kind="ExternalInput").ap()
    c1_d = nc.dram_tensor("c1", [1, H], f32, kind="ExternalInput").ap()
    lng_d = nc.dram_tensor("ln_g", [H, 1], f32, kind="ExternalInput").ap()
    lnb_d = nc.dram_tensor("ln_b", [H, 1], f32, kind="ExternalInput").ap()
    bfc_d = nc.dram_tensor("bfc", [1, NUM_CLASSES], f32, kind="ExternalInput").ap()
    iota_d = nc.dram_tensor("iota", [1, P], f16, kind="ExternalInput").ap()
    idm_d = nc.dram_tensor("idm", [P, P], f16, kind="ExternalInput").ap()
    out_d = nc.dram_tensor("out", [TPC * P, NUM_CLASSES], f32,
                           kind="ExternalOutput").ap()

    def bcast(src_ap, parts=P):
        return bass.AP(tensor=src_ap.tensor, offset=src_ap.offset,
                       ap=[[0, parts]] + list(src_ap.ap[1:]))

    def rep_mid(ap2d, n):
        """[p, q] -> [p, n, q] with the middle dim broadcast (stride 0)."""
        a = list(ap2d.ap)
        return bass.AP(tensor=ap2d.tensor, offset=ap2d.offset,
                       ap=[a[0], [0, n], a[1]])

    # 4D APs for the one-hot slab build; every operand keeps an innermost
    # stride-1 dim of size 2 so the DVE can enter 2x_1P perf mode.
    def meta_rep4(ap2d):
        """paired meta [p, 2*GCH] -> [p, GCH, 64, 2]; value const along dim 64."""
        a = list(ap2d.ap)
        return bass.AP(tensor=ap2d.tensor, offset=ap2d.offset,
                       ap=[a[0], [2, GCH], [0, P // 2], [1, 2]])

    def iota_rep4(ap2d):
        """IOTA [p, 128] -> [p, GCH, 64, 2]; iota along the last two dims."""
        a = list(ap2d.ap)
        return bass.AP(tensor=ap2d.tensor, offset=ap2d.offset,
                       ap=[a[0], [0, GCH], [2, P // 2], [1, 2]])

    def slab4(ap2d):
        """slab [p, GCH*128] -> [p, GCH, 64, 2] contiguous."""
        a = list(ap2d.ap)
        return bass.AP(tensor=ap2d.tensor, offset=ap2d.offset,
                       ap=[a[0], [P, GCH], [2, P // 2], [1, 2]])

    AL = mybir.AluOpType
    AF = mybir.ActivationFunctionType

    with tile.TileContext(nc) as tc, ExitStack() as ctx:
        consts = ctx.enter_context(tc.tile_pool(name="consts", bufs=1))
        gpool = ctx.enter_context(tc.tile_pool(name="gather", bufs=2))
        ohp = ctx.enter_context(tc.tile_pool(name="onehot", bufs=3))
        sp = ctx.enter_context(tc.tile_pool(name="work", bufs=4))
        t1p = ctx.enter_context(tc.tile_pool(name="t1c", bufs=2 * GS))
        statp = ctx.enter_context(tc.tile_pool(name="stats", bufs=4 * GS))
        pp_ps = ctx.enter_context(tc.tile_pool(name="pp_ps", bufs=2, space="PSUM"))
        agg_ps = ctx.enter_context(tc.tile_pool(name="agg_ps", bufs=2, space="PSUM"))
        tr_ps = ctx.enter_context(tc.tile_pool(name="tr_ps", bufs=2, space="PSUM"))
        fc_ps = ctx.enter_context(tc.tile_pool(name="fc_ps", bufs=2, space="PSUM"))

        W1s = consts.tile([IN_CH, H + 1], f16)
        nc.sync.dma_start(W1s[:], w1_d[:])
        Wfcs = consts.tile([H, NUM_CLASSES], f16)
        nc.sync.dma_start(Wfcs[:], wfc_d[:])
        C1s = consts.tile([P, H], f32)
        nc.sync.dma_start(C1s[:], bcast(c1_d))
        Gcol = consts.tile([H, 1], f32)
        nc.sync.dma_start(Gcol[:], lng_d[:])
        Bcol = consts.tile([H, 1], f32)
        nc.sync.dma_start(Bcol[:], lnb_d[:])
        BFCs = consts.tile([P, NUM_CLASSES], f32)
        nc.sync.dma_start(BFCs[:], bcast(bfc_d))
        IOTA = consts.tile([P, P], f16)
        nc.sync.dma_start(IOTA[:], bcast(iota_d))
        idents = consts.tile([P, P], f16)
        nc.sync.dma_start(idents[:], idm_d[:])
        eps_t = consts.tile([P, 1], f32)
        nc.vector.memset(eps_t[:], LN_EPS)

        idx_s = consts.tile([P, IDXC], i16)
        nc.sync.dma_start(idx_s[:], idx_d[:])
        dstl_s = consts.tile([P, MC], f16)
        nc.sync.dma_start(dstl_s[:], dstl_d[:])
        norm_s = consts.tile([P, MC], f16)
        nc.sync.dma_start(norm_s[:], norm_d[:])
        selfoh_s = consts.tile([P, TPC * P], f16)
        nc.sync.dma_start(selfoh_s[:], selfoh_d[:])

        out_acc = consts.tile([P, TPC * NUM_CLASSES], f32)

        # Software-pipelined emission: the PE stream is kept dense by skewing
        # every cross-engine round trip behind enough chunk-matmul work that
        # its dependencies are already resolved when the PE (or DVE/ACT FIFO
        # head) reaches it.
        def emit_agg(st):
            agg = agg_ps.tile([P, H + 1], f32, space="PSUM")
            nc.tensor.matmul(agg[:], lhsT=st["Ps"][:], rhs=W1s[:],
                             start=True, stop=True)
            st["agg"] = agg

        def emit_stats(st):
            agg = st["agg"]
            mu = statp.tile([P, 1], f32, tag="mu")
            nc.scalar.activation(out=mu[:], in_=agg[:, H:H + 1], func=AF.Copy)
            t1c = t1p.tile([P, H], f32, tag="t1c")
            nc.vector.scalar_tensor_tensor(
                out=t1c[:], in0=agg[:, 0:H], scalar=mu[:], in1=C1s[:],
                op0=AL.add, op1=AL.add)
            sq = sp.tile([P, H], f32, tag="sq")
            nc.scalar.activation(out=sq[:], in_=t1c[:], func=AF.Square,
                                 accum_out=st["vars"][:, st["j"]:st["j"] + 1])
            st["t1c"] = t1c

        def emit_b1(st):
            t1n = sp.tile([P, H], f16, tag="t1n")
            nc.scalar.activation(out=t1n[:], in_=st["t1c"][:], func=AF.Copy,
                                 scale=st["rstd"][:, st["j"]:st["j"] + 1])
            yT = tr_ps.tile([H, P], f16, space="PSUM")
            nc.tensor.transpose(out=yT[:], in_=t1n[:], identity=idents[:])
            st["yT"] = yT

        def emit_b2(st):
            hrT = sp.tile([H, P], f16, tag="hrT")
            nc.scalar.activation(out=hrT[:], in_=st["yT"][:], func=AF.Relu,
                                 scale=Gcol[:], bias=Bcol[:])
            j = st["j"]
            nc.tensor.matmul(st["fc"][:, j * NUM_CLASSES:(j + 1) * NUM_CLASSES],
                             lhsT=hrT[:], rhs=Wfcs[:], start=True, stop=True)

        def emit_outadd(gst):
            s, tb, fc = gst["s"], gst["tb"], gst["fc"]
            oslice = out_acc[:, tb * NUM_CLASSES:(tb + s) * NUM_CLASSES]
            o3 = oslice.rearrange("p (t c) -> p t c", c=NUM_CLASSES)
            f3 = fc[:, 0:s * NUM_CLASSES].rearrange("p (t c) -> p t c",
                                                    c=NUM_CLASSES)
            nc.vector.tensor_tensor(out=o3, in0=f3,
                                    in1=rep_mid(BFCs[:], s), op=AL.add)

        prev_states = None   # tile states of the previous group (pass B pending)
        prev_gst = None
        for gi, (s, tb) in enumerate(_groups()):
            Gg = gpool.tile([P, s * CPT, IN_CH], f16, tag="Gg")
            for b in range(NBANK):
                n = s * K * P
                cbase = bases[gi][b]
                nc.gpsimd.dma_gather(
                    out_ap=Gg[:, b * s * K:(b + 1) * s * K, :],
                    in_ap=xb[b][:],
                    idxs_ap=idx_s[:, cbase:cbase + n // 16],
                    num_idxs=n, num_idxs_reg=n, elem_size=IN_CH,
                    single_packet=False, queue_num=b,
                )
            # dense self blocks via HWDGE
            xp_in = bass.AP(tensor=xperm_d.tensor,
                            offset=xperm_d.offset + tb * P * IN_CH,
                            ap=[[IN_CH, P], [P * IN_CH, s], [1, IN_CH]])
            nc.sync.dma_start(Gg[:, GCH * s:GCH * s + s, :], xp_in)

            vars_g = statp.tile([P, s], f32, tag="vars")
            states = []
            for j in range(s):
                t = tb + j
                mc0 = t * GCH * 2
                dsl = dstl_s[:, mc0:mc0 + GCH * 2]
                nsl = norm_s[:, mc0:mc0 + GCH * 2]
                tmp = ohp.tile([P, GCH * P], f16, tag="tmp")
                nc.vector.tensor_tensor(out=slab4(tmp[:]), in0=meta_rep4(dsl),
                                        in1=iota_rep4(IOTA[:]), op=AL.is_equal)
                ohs = ohp.tile([P, GCH * P], f16, tag="ohs")
                nc.vector.tensor_tensor(out=slab4(ohs[:]), in0=slab4(tmp[:]),
                                        in1=meta_rep4(nsl), op=AL.mult)

                Pp = pp_ps.tile([IN_CH, P], f32, space="PSUM")
                for c in range(CPT):
                    if c < GCH:
                        gcol = (c // K) * s * K + j * K + (c % K)
                        rhs = ohs[:, c * P:(c + 1) * P]
                    else:
                        gcol = GCH * s + j
                        rhs = selfoh_s[:, t * P:(t + 1) * P]
                    nc.tensor.matmul(Pp[:], lhsT=Gg[:, gcol, :], rhs=rhs,
                                     start=(c == 0), stop=(c == CPT - 1))
                Ps = sp.tile([IN_CH, P], f16, tag="Ps")
                nc.scalar.activation(out=Ps[:], in_=Pp[:], func=AF.Copy)
                states.append({"j": j, "Ps": Ps, "vars": vars_g})

                # skewed tails: agg one tile behind, stats two tiles behind
                if j >= 1:
                    emit_agg(states[j - 1])
                if j >= 2:
                    emit_stats(states[j - 2])
                # previous group's pass B: b1 at tile j, b2 one tile later
                if prev_states is not None:
                    if j < len(prev_states):
                        emit_b1(prev_states[j])
                    if 1 <= j <= len(prev_states):
                        emit_b2(prev_states[j - 1])

            # group-boundary flush of pass A
            emit_agg(states[s - 1])
            for jj in range(max(0, s - 2), s):
                emit_stats(states[jj])
            if prev_states is not None:
                for k in range(s, len(prev_states)):
                    emit_b1(prev_states[k])
                for k in range(max(1, s), len(prev_states) + 1):
                    emit_b2(prev_states[k - 1])
                emit_outadd(prev_gst)

            stdg = statp.tile([P, s], f32, tag="std")
            nc.scalar.activation(out=stdg[:], in_=vars_g[:], func=AF.Sqrt,
                                 bias=eps_t[:], scale=1.0 / H)
            rstd = statp.tile([P, s], f32, tag="rstd")
            nc.vector.reciprocal(out=rstd[:], in_=stdg[:])
            fc = fc_ps.tile([P, s * NUM_CLASSES], f32, space="PSUM")
            for st in states:
                st["rstd"] = rstd
                st["fc"] = fc
            prev_states = states
            prev_gst = {"s": s, "tb": tb, "fc": fc}

        # final group's pass B
        for st in prev_states:
            emit_b1(st)
            emit_b2(st)
        emit_outadd(prev_gst)

        out_view = out_d.rearrange("(t p) c -> p t c", p=P)
        acc_view = out_acc[:].rearrange("p (t c) -> p t c", c=NUM_CLASSES)
        nc.sync.dma_start(out_view, acc_view)

    nc.compile()
    return nc


def _ensure_ntff_hook():
    import sys, types
    try:
        from antenv.axon_hooks import get_axon_ntff_profile_hook  # noqa: F401
        return
    except ImportError:
        pass
    mod = types.ModuleType("antenv.axon_hooks")
    _hook = [None]
    mod.set_axon_ntff_profile_hook = lambda h: _hook.__setitem__(0, h)
    mod.get_axon_ntff_profile_hook = lambda: _hook[0]
    sys.modules["antenv.axon_hooks"] = mod
    try:
        import antenv
        antenv.axon_hooks = mod
    except ImportError:
        pass
    try:
        from trn_agent_boot.trn_boot import _ntff_profile_via_ctypes
        mod.set_axon_ntff_profile_hook(
            _ntff_profile_via_ctypes("/opt/axon/libaxon_pjrt.so"))
    except Exception:
        pass


# ----------------------------------------------------------------------------
# entry point
# ----------------------------------------------------------------------------
def kernel(x, edge_index, edge_weight, W1, b1, ln_g, ln_b, Wfc, bfc):
    global LAST_RESULTS
    from concourse.bass_utils import run_bass_kernel_spmd

    x16 = np.asarray(x, dtype=np.float32).astype(np.float16)
    meta = _preprocess(edge_index, edge_weight)
    IDXC = meta["IDXC"]

    if "prog" not in _PROGRAM_CACHE:
        _PROGRAM_CACHE["prog"] = _build_program()
    nc = _PROGRAM_CACHE["prog"]

    W1f = np.asarray(W1, np.float32)
    W1aug = np.zeros((IN_CH, HIDDEN + 1), dtype=np.float16)
    W1aug[:, :HIDDEN] = W1f.astype(np.float16)
    W1aug[:, HIDDEN] = (-W1f.mean(axis=1)).astype(np.float16)
    b1f = np.asarray(b1, np.float32).reshape(-1)
    c1 = (b1f - b1f.mean()).reshape(1, HIDDEN).astype(np.float32)

    xperm = np.zeros((TILES * P, IN_CH), dtype=np.float16)
    rows = meta["node_tile"] * P + meta["node_slot"]
    xperm[rows] = x16

    banks = {}
    for b in range(NBANK):
        blk = np.zeros((BANK, IN_CH), dtype=np.float16)
        seg = x16[b * BANK:(b + 1) * BANK]
        blk[:len(seg)] = seg
        banks[f"xb{b}"] = blk

    common = dict(
        banks,
        W1aug=W1aug,
        Wfc=np.asarray(Wfc, np.float32).astype(np.float16),
        c1=c1,
        ln_g=np.asarray(ln_g, np.float32).reshape(HIDDEN, 1),
        ln_b=np.asarray(ln_b, np.float32).reshape(HIDDEN, 1),
        bfc=np.asarray(bfc, np.float32).reshape(1, NUM_CLASSES),
        iota=np.arange(P, dtype=np.float16).reshape(1, P),
        idm=np.eye(P, dtype=np.float16),
    )
    MC = TPC * NBANK * K * 2
    in_maps = []
    for core in range(N_CORES):
        msl = slice(core * MC, (core + 1) * MC)
        ssl = slice(core * TPC * P, (core + 1) * TPC * P)
        in_maps.append(dict(
            common,
            idx=np.ascontiguousarray(meta["idx_all"][:, core * IDXC:(core + 1) * IDXC]),
            dstl=np.ascontiguousarray(meta["dstl_all"][:, msl]),
            normv=np.ascontiguousarray(meta["norm_all"][:, msl]),
            selfoh=np.ascontiguousarray(meta["selfoh"][:, ssl]),
            xperm=np.ascontiguousarray(xperm[ssl]),
        ))

    trace = bool(os.environ.get("KERNEL_TRACE"))
    if trace:
        _ensure_ntff_hook()
    res = run_bass_kernel_spmd(nc, in_maps, list(range(N_CORES)), trace=trace)
    LAST_RESULTS = res

    all_rows = np.concatenate([res.results[c]["out"] for c in range(N_CORES)],
                              axis=0)
    return np.ascontiguousarray(all_rows[rows].astype(np.float32))
